# revision 29
# baseline (speedup 1.0000x reference)
"""Grouped gated DeltaNet (KDA-style) on 8 TRN2 NeuronCores — v3.

Sharding: core c -> (batch b = c//4, head-group hg = c%4 of 4 heads).

v3 restructure vs v2:
- single software-pipelined emission stream: half-1 projections, the
  output projection, and DMA-out are drained into the chunk recurrence
  via a work queue (no sequential phase barriers)
- head-stacked gate path: one [32,T] gna tile, one cN scan per chunk,
  [64,C] hi/lo decay tile with per-head selector consts
- decay-difference build folded: stacked hi+lo contraction (1+4 matmuls
  per head-half instead of 2+8)
- conv in bf16 (DVE 2x mode); act-table switches confined to two silu
  batches (tanh-based sigmoid shares the silu table)
- weights streamed per half through a 2-deep ring (SBUF fit)
- per-half output projection pieces overlap the second half of the
  recurrence; DMA-out streams during compute

Self-contained: B=2, T=1024, D=2048, H=16, DK=DV=128 hardcoded.
"""
import sys
sys.path.insert(0, '/opt/trn_rl_repo')
import numpy as np
import ml_dtypes
from contextlib import ExitStack

B, T, D = 2, 1024, 2048
H, DK, DV, GG = 16, 128, 128, 16
NG = DK // GG          # 8 gate groups per head
NH = 4                 # heads per core
C = 128                # chunk length
NCH = T // C
NLEV = 2               # Neumann doubling levels (covers N^k, k < 2^NLEV)
SCALE = DK ** -0.5
EPS = 1e-5

BF = ml_dtypes.bfloat16
_CACHE = {}


def _build():
    import concourse.tile as tile
    from concourse import bacc, mybir

    fp32 = mybir.dt.float32
    bf16 = mybir.dt.bfloat16
    Alu = mybir.AluOpType
    Act = mybir.ActivationFunctionType

    nc = bacc.Bacc("TRN2", target_bir_lowering=False, debug=False, num_devices=8)
    dp = lambda n, sh, dt: nc.dram_tensor(n, sh, dt, kind="ExternalInput").ap()
    hT = dp("hT", [D, T], bf16)
    wqkvg = dp("wqkvg", [D, 4 * NH * DK], bf16)
    wo = dp("wo", [NH * DV, D], bf16)
    wf1 = dp("wf1", [D, DV], bf16)
    wf2 = dp("wf2", [DV, NH * NG], bf16)
    wb = dp("wb", [D, NH], bf16)
    cw = dp("cw", [NH * DK, 12], fp32)
    nega32 = dp("nega32", [32, 1], fp32)
    dtb32 = dp("dtb32", [32, 1], fp32)
    bgc = dp("bgc", [DV, NH], fp32)
    normw = dp("normw", [DV, 1], fp32)
    repl64 = dp("repl64", [64, 4 * DK], bf16)
    s64f = dp("s64f", [64, 4 * NG * C], bf16)
    oh4 = dp("oh4", [DK, 16], bf16)
    oh4b = dp("oh4b", [4, 4 * DK], bf16)
    evodb = dp("evodb", [DK, 4 * C], bf16)
    oh8 = dp("oh8", [DK, 64], bf16)
    sel8b = dp("sel8b", [8, 8 * 128], bf16)
    sc8 = dp("sc8", [8, 1], fp32)
    eps8 = dp("eps8", [8, 1], fp32)
    nmaskM = dp("nmaskM", [C, C], bf16)   # -1 strictly upper (s<t)
    maskG = dp("maskG", [C, C], bf16)     # +1 upper incl diag (s<=t)
    idbf = dp("idbf", [128, 128], bf16)
    outT = nc.dram_tensor("outT", [D, T], fp32, kind="ExternalOutput").ap()
    DBG = bool(__import__('os').environ.get('K3_DEBUG'))
    dbg = {}
    if DBG:
        for nm in ['d_qb', 'd_kb', 'd_vb', 'd_gateb', 'd_yf']:
            for m in range(4):
                dbg[f'{nm}{m}'] = nc.dram_tensor(
                    f'{nm}{m}', [128, T], bf16, kind="ExternalOutput").ap()
        dbg['d_gna'] = nc.dram_tensor('d_gna', [32, T], fp32,
                                      kind="ExternalOutput").ap()
        dbg['d_bsg'] = nc.dram_tensor('d_bsg', [4, T], bf16,
                                      kind="ExternalOutput").ap()
        dbg['d_sqs'] = nc.dram_tensor('d_sqs', [4, T], fp32,
                                      kind="ExternalOutput").ap()

    with tile.TileContext(nc) as tc, ExitStack() as ctx:
        pool = lambda name, bufs, space="SBUF": ctx.enter_context(
            tc.tile_pool(name=name, bufs=bufs, space=space))

        cons = pool("cons", 1)
        pers = pool("pers", 1)
        st = pool("st", 1)
        wk = pool("wk", 2)
        pr = pool("pr", 1, "PSUM")

        dma = nc.sync.dma_start

        # ---- conv weights first (needed ~25us in), then the h/q-weight
        #      stream; everything else queues behind ----
        cwt = []
        for m in range(4):
            t = cons.tile([128, 12], fp32, tag=f"cw{m}", name=f"cw{m}")
            dma(t[:], cw[m * 128:(m + 1) * 128, :])
            cwt.append(t)

        ht = []
        wtiles = {}

        def wslot(k):
            return wk.tile([128, 512], bf16, tag=f"w{k}", name=f"w{k}",
                           bufs=1)

        def load_wset(proj):
            wt = [wslot(k) for k in range(16)]
            for k in range(16):
                dma(wt[k][:], wqkvg[k * 128:(k + 1) * 128,
                                    proj * 512:(proj + 1) * 512])
            wtiles[proj] = wt

        wt0 = [wslot(k) for k in range(16)]
        for k in range(16):
            t = pers.tile([128, T], bf16, tag=f"ht{k}", name=f"ht{k}")
            dma(t[:], hT[k * 128:(k + 1) * 128, :])
            dma(wt0[k][:], wqkvg[k * 128:(k + 1) * 128, 0:512])
            ht.append(t)
        wtiles[0] = wt0

        # ---- remaining weights + consts (arrive mid-segment-A) ----
        wf1t = [cons.tile([128, 128], bf16, tag=f"wf1_{k}", name=f"wf1_{k}")
                for k in range(16)]
        wbt = [cons.tile([128, 4], bf16, tag=f"wb{k}", name=f"wbt{k}")
               for k in range(16)]
        for k in range(16):
            dma(wf1t[k][:], wf1[k * 128:(k + 1) * 128, :])
            dma(wbt[k][:], wb[k * 128:(k + 1) * 128, :])

        def ctile(shape, dt, src, nm):
            t = cons.tile(shape, dt, tag=nm, name=nm)
            dma(t[:], src[:])
            return t
        wf2t = ctile([128, 32], bf16, wf2, "wf2t")
        negat = ctile([32, 1], fp32, nega32, "negat")
        dtbt = ctile([32, 1], fp32, dtb32, "dtbt")
        bgt = ctile([128, 4], fp32, bgc, "bgt")
        nwt = ctile([128, 1], fp32, normw, "nwt")
        idb = ctile([128, 128], bf16, idbf, "idb")
        r64t = ctile([64, 4 * 128], bf16, repl64, "r64t")
        s64c = ctile([64, 4 * NG * C], bf16, s64f, "s64c")
        oh4t = ctile([128, 16], bf16, oh4, "oh4t")
        oh4bt = ctile([4, 4 * 128], bf16, oh4b, "oh4bt")
        evt = ctile([128, 4 * C], bf16, evodb, "evt")
        oh8t = ctile([128, 64], bf16, oh8, "oh8t")
        s8b = ctile([8, 8 * 128], bf16, sel8b, "s8b")
        sc8t = ctile([8, 1], fp32, sc8, "sc8t")
        eps8t = ctile([8, 1], fp32, eps8, "eps8t")
        nmM = ctile([128, 128], bf16, nmaskM, "nmM")
        mGt = ctile([128, 128], bf16, maskG, "mGt")
        ones32 = cons.tile([32, C], fp32, tag="ones32", name="ones32")
        nc.vector.memset(ones32[:], 1.0)
        eps4 = cons.tile([4, 1], fp32, tag="eps4", name="eps4")
        nc.vector.memset(eps4[:], EPS)
        neg4c = cons.tile([128, 4 * C], bf16, tag="neg4c", name="neg4c")
        nc.vector.memset(neg4c[:], -1.0)

        # ---- persistent activations ----
        mk = lambda p, nm: [p.tile([128, T], bf16, tag=f"{nm}{m}",
                                   name=f"{nm}{m}") for m in range(4)]
        qb, kb, vb = mk(pers, "qb"), mk(pers, "kb"), mk(pers, "vb")
        gateb, yfall = mk(pers, "gateb"), mk(pers, "yfall")
        gna32 = cons.tile([32, T], fp32, tag="gna32", name="gna32")
        bsg = cons.tile([4, T], bf16, tag="bsg", name="bsg")
        sqs = cons.tile([4, T], fp32, tag="sqs", name="sqs")
        f1b = cons.tile([128, T], bf16, tag="f1b", name="f1b")

        # ---- PSUM rings: pp x4 (proj/bca/pall/red/ssq/out), xaq x1,
        #      q32a x2, q16b x1 ----
        def pp(nm):
            return pr.tile([128, 4 * C], fp32, tag="pp", bufs=4, name=nm)

        def q32(nm):
            return pr.tile([128, 4 * C], fp32, tag="q32a", bufs=2, name=nm)

        def q16(nm):
            return pr.tile([128, 4 * C], bf16, tag="q16b", bufs=1, name=nm)

        # ---- projection pieces ----
        accs = {}       # (proj, m) -> conv accumulator [128, 512]
        bnds = {}       # (proj, m) -> 3-col conv boundary
        gcs = {}        # m -> gate pre-silu copy

        def proj_piece(proj, m, half):
            """16 matmuls; conv projs: PSUM->xpad copy + 4 DVE taps into
            acc; gate proj: PSUM->SBUF copy. Silu deferred to a batch."""
            wt = wtiles[proj]
            sl = slice(half * 512, (half + 1) * 512)
            ps = pp(f"prj{proj}{m}{half}")
            for k in range(16):
                nc.tensor.matmul(ps[:, 0:512], wt[k][:, m * 128:(m + 1) * 128],
                                 ht[k][:, sl], start=(k == 0), stop=(k == 15))
            if proj == 3:
                gc = wk.tile([128, 512], bf16, tag=f"ac1{m}", name=f"gc{m}",
                             bufs=1)
                nc.scalar.copy(gc[:], ps[:, 0:512])
                gcs[m] = gc
                return
            xpad = wk.tile([128, 515], bf16, tag="xpad", name="xpad", bufs=2)
            if half == 0:
                nc.vector.memset(xpad[:, 0:3], 0.0)
            else:
                nc.vector.tensor_copy(xpad[:, 0:3], bnds[(proj, m)][:])
            nc.scalar.copy(xpad[:, 3:515], ps[:, 0:512])
            if half == 0:
                bnd = wk.tile([128, 3], bf16, tag=f"bnd{proj}{m}",
                              name=f"bnd{proj}{m}", bufs=1)
                nc.vector.tensor_copy(bnd[:], xpad[:, 512:515])
                bnds[(proj, m)] = bnd
            cwm = cwt[m]
            s = proj * 4
            a = wk.tile([128, 512], bf16, tag=f"ac{proj % 2}{m}",
                        name=f"ac{proj}{m}", bufs=1)
            b2 = wk.tile([128, 512], bf16, tag="acw", name="acw", bufs=2)
            nc.vector.tensor_scalar(b2[:], xpad[:, 3:515],
                                    cwm[:, s + 3:s + 4], None, op0=Alu.mult)
            cur, nxt = b2, a
            for kk in (2, 1, 0):
                nc.vector.scalar_tensor_tensor(
                    nxt[:], xpad[:, kk:kk + 512], cwm[:, s + kk:s + kk + 1],
                    cur[:], op0=Alu.mult, op1=Alu.add)
                cur, nxt = nxt, cur
            accs[(proj, m)] = a      # 3 stt steps end in `a`

        def silu_qk(half):
            sl = slice(half * 512, (half + 1) * 512)
            for m in range(4):
                nc.scalar.activation(qb[m][:, sl], accs[(0, m)][:], Act.Silu)
            for m in range(4):
                nc.scalar.activation(kb[m][:, sl], accs[(1, m)][:], Act.Silu)

        def silu_vg(half):
            sl = slice(half * 512, (half + 1) * 512)
            for m in range(4):
                nc.scalar.activation(vb[m][:, sl], accs[(2, m)][:], Act.Silu)
            for m in range(4):
                nc.scalar.activation(gateb[m][:, sl], gcs[m][:], Act.Silu,
                                     bias=bgt[:, m:m + 1])

        def beta_piece():
            for half in range(2):
                bps = pp(f"bps{half}")
                for k in range(16):
                    nc.tensor.matmul(bps[0:4, 0:512], wbt[k][:],
                                     ht[k][:, half * 512:(half + 1) * 512],
                                     start=(k == 0), stop=(k == 15))
                # sigmoid(x) = 0.5 + 0.5*tanh(x/2) (shares the silu table)
                tb = wk.tile([4, 512], bf16, tag="tb", name=f"tb{half}")
                nc.scalar.activation(tb[:], bps[0:4, 0:512], Act.Tanh,
                                     scale=0.5)
                nc.vector.tensor_scalar(bsg[:, half * 512:(half + 1) * 512],
                                        tb[:], 0.5, 0.5, op0=Alu.mult,
                                        op1=Alu.add)

        def fgate_piece():
            """f1 projection + grouped softplus gate, both halves (Exp/Ln)."""
            for half in range(2):
                sl = slice(half * 512, (half + 1) * 512)
                ps = pp(f"f1p{half}")
                for k in range(16):
                    nc.tensor.matmul(ps[:, 0:512], wf1t[k][:], ht[k][:, sl],
                                     start=(k == 0), stop=(k == 15))
                nc.scalar.copy(f1b[:, sl], ps[:, 0:512])
            for half in range(2):
                sl = slice(half * 512, (half + 1) * 512)
                gp = pp(f"gp{half}")
                nc.tensor.matmul(gp[0:32, 0:512], wf2t[:, 0:32], f1b[:, sl],
                                 start=True, stop=True)
                spe = wk.tile([32, 512], fp32, tag="spe", name=f"spe{half}", bufs=1)
                nc.scalar.activation(spe[:], gp[0:32, 0:512], Act.Exp,
                                     bias=dtbt[:, 0:1])
                sp = wk.tile([32, 512], fp32, tag="spx", name=f"sp{half}", bufs=1)
                nc.scalar.activation(sp[:], spe[:], Act.Ln,
                                     bias=ones32[:, 0:1])
                nc.vector.tensor_scalar(gna32[:, sl], sp[:], negat[:, 0:1],
                                        None, op0=Alu.mult)

        def norm_piece(half, grp):
            """l2 normalizers for pairs grp*4..grp*4+3 (q: grp 0, k: grp 1)."""
            sl = slice(half * 512, (half + 1) * 512)
            rows = slice(grp * 4, grp * 4 + 4)
            ssq = pp(f"ssq{half}{grp}")
            for i in range(4):
                p = grp * 4 + i
                src = qb[p][:, sl] if p < 4 else kb[p - 4][:, sl]
                sq = wk.tile([128, 512], bf16, tag="sq", name=f"sq{p}{half}")
                nc.vector.tensor_tensor(sq[:], src, src, op=Alu.mult)
                nc.tensor.matmul(ssq[0:8, 0:512], oh8t[:, p * 8:p * 8 + 8],
                                 sq[:], start=(i == 0), stop=(i == 3))
            nrm = wk.tile([8, 512], fp32, tag="nrm", name=f"nrm{half}{grp}", bufs=1)
            nc.scalar.activation(nrm[:], ssq[0:8, 0:512], Act.Ln,
                                 scale=sc8t[:, 0:1], bias=eps8t[:, 0:1])
            recb = wk.tile([8, 512], bf16, tag="recb", name=f"rec{half}{grp}")
            nc.scalar.activation(recb[:], nrm[:], Act.Exp, scale=-0.5)
            for i in range(4):
                p = grp * 4 + i
                dst = qb[p][:, sl] if p < 4 else kb[p - 4][:, sl]
                nb = pp(f"nb{p}{half}")
                nc.tensor.matmul(nb[:, 0:512], s8b[:, p * 128:(p + 1) * 128],
                                 recb[:], start=True, stop=True)
                nc.vector.tensor_tensor(dst, dst, nb[:, 0:512], op=Alu.mult)

        # ---- output projection pieces (per half) ----
        wot = [pers.tile([128, D], bf16, tag=f"wo{k}", name=f"wo{k}")
               for k in range(4)]

        def wo_piece():
            for k in range(4):
                dma(wot[k][:], wo[k * 128:(k + 1) * 128, :])

        def onorm_piece(half):
            sl = slice(half * 512, (half + 1) * 512)
            nrm4 = wk.tile([4, 512], fp32, tag="nrm4", name=f"nrm4{half}")
            nc.scalar.activation(nrm4[:], sqs[:, sl], Act.Ln, scale=1.0 / DV,
                                 bias=eps4[:, 0:1])
            rst4 = wk.tile([4, 512], bf16, tag="rst4", name=f"rst4{half}")
            nc.scalar.activation(rst4[:], nrm4[:], Act.Exp, scale=-0.5)
            for h in range(4):
                rbc = pp(f"rbc{h}_{half}")
                nc.tensor.matmul(rbc[:, 0:512], oh4bt[:, h * 128:(h + 1) * 128],
                                 rst4[:], start=True, stop=True)
                nc.vector.scalar_tensor_tensor(yfall[h][:, sl], yfall[h][:, sl],
                                               nwt[:, 0:1], rbc[:, 0:512],
                                               op0=Alu.mult, op1=Alu.mult)

        def om_piece(half, m):
            sl = slice(half * 512, (half + 1) * 512)
            ps = pp(f"ops{m}_{half}")
            for k in range(4):
                nc.tensor.matmul(ps[:, 0:512], wot[k][:, m * 128:(m + 1) * 128],
                                 yfall[k][:, sl], start=(k == 0), stop=(k == 3))
            osb = wk.tile([128, 512], fp32, tag="osb", name=f"osb{m}{half}",
                          bufs=2)
            if m % 2 == 0:
                nc.vector.tensor_copy(osb[:], ps[:, 0:512])
            else:
                nc.scalar.copy(osb[:], ps[:, 0:512])
            dma(outT[m * 128:(m + 1) * 128, sl], osb[:])

        # ---- recurrence (stage_a / stage_b) ----
        Sf = [st.tile([128, 128], fp32, tag=f"Sf{h}", name=f"Sf{h}")
              for h in range(4)]
        Sb = [st.tile([128, 128], bf16, tag=f"Sb{h}", name=f"Sb{h}")
              for h in range(4)]
        for h in range(4):
            nc.vector.memset(Sf[h][:], 0.0)
            nc.vector.memset(Sb[h][:], 0.0)

        ax_store = {}

        def _a_prep(ci):
            ts = slice(ci * C, (ci + 1) * C)
            prep = pr.tile([128, 4], bf16, tag="q32a", bufs=2,
                           name=f"prep{ci}")
            nc.tensor.transpose(prep[:], bsg[:, ts], idb[0:4, 0:4])
            beta2 = wk.tile([128, 4], fp32, tag="beta2", name=f"beta2_{ci}")
            nc.scalar.copy(beta2[:], prep[:])
            cN32 = wk.tile([32, C], fp32, tag="cN32", name=f"cN32_{ci}")
            nc.vector.tensor_tensor_scan(cN32[:], ones32[:], gna32[:, ts],
                                         0.0, op0=Alu.mult, op1=Alu.add)
            c64 = wk.tile([64, C], bf16, tag="c64", name=f"c64_{ci}")
            nc.scalar.copy(c64[0:32, :], cN32[:])
            nc.vector.tensor_tensor(c64[32:64, :], cN32[:], c64[0:32, :],
                                    op=Alu.subtract)
            n64 = wk.tile([64, C], bf16, tag="n64", name=f"n64_{ci}")
            nc.gpsimd.tensor_tensor(n64[:], c64[:], neg4c[0:64, 0:C],
                                    op=Alu.mult)

            # channel decay expansion, all heads in one quad
            cfq = q32(f"cfq{ci}")
            for h in range(4):
                hs_ = slice(h * C, (h + 1) * C)
                nc.tensor.matmul(cfq[:, hs_], r64t[:, h * 128:(h + 1) * 128],
                                 c64[:], start=True, stop=True)
            nclq = wk.tile([128, 4], fp32, tag="nclq", name=f"nclq{ci}")
            for h in range(4):
                nc.vector.tensor_scalar(nclq[:, h:h + 1],
                                        cfq[:, h * C + C - 1:h * C + C],
                                        -1.0, None, op0=Alu.mult)
            bfq = wk.tile([128, 4 * C], bf16, tag="bfq", name=f"bfq{ci}", bufs=1)
            nc.scalar.activation(bfq[:], cfq[:], Act.Exp, scale=-1.0)
            kfq = wk.tile([128, 4 * C], bf16, tag="kfq", name=f"kfq{ci}", bufs=1)
            for h in range(4):
                hs_ = slice(h * C, (h + 1) * C)
                nc.scalar.activation(kfq[:, hs_], cfq[:, hs_], Act.Exp,
                                     bias=nclq[:, h:h + 1])
            bCq = wk.tile([128, 4], fp32, tag="bCq", name=f"bCq{ci}")
            nc.scalar.activation(bCq[:], nclq[:], Act.Exp)
            nbfq = wk.tile([128, 4 * C], bf16, tag="nbfq", name=f"nbfq{ci}", bufs=1)
            nc.gpsimd.tensor_tensor(nbfq[:], bfq[:], neg4c[:], op=Alu.mult)

            # decayed k/q streams (Pool)
            negWt, qtT, kend = [], [], []
            for h in range(4):
                hs_ = slice(h * C, (h + 1) * C)
                nw = wk.tile([128, C], bf16, tag=f"negWt{h}",
                             name=f"negWt{h}_{ci}")
                nc.gpsimd.tensor_tensor(nw[:], kb[h][:, ts], nbfq[:, hs_],
                                        op=Alu.mult)
                qt = wk.tile([128, C], bf16, tag=f"qtT{h}", name=f"qtT{h}_{ci}")
                nc.gpsimd.tensor_tensor(qt[:], qb[h][:, ts], bfq[:, hs_],
                                        op=Alu.mult)
                ke = wk.tile([128, C], bf16, tag=f"kend{h}",
                             name=f"kend{h}_{ci}")
                nc.gpsimd.tensor_tensor(ke[:], kb[h][:, ts], kfq[:, hs_],
                                        op=Alu.mult)
                negWt.append(nw); qtT.append(qt); kend.append(ke)

            ealls = [None] * 4

            def corr(h, srcq, mask_t, scale_col, nm, dst):
                kms = []
                for j in range(4):
                    km = wk.tile([128, C], bf16, tag="km",
                                 name=f"km{j}_{h}_{nm}_{ci}", bufs=4)
                    nc.gpsimd.tensor_tensor(km[:], kb[h][:, ts],
                                            evt[:, j * C:(j + 1) * C],
                                            op=Alu.mult)
                    kms.append(km)
                prods = []
                for half in range(2):
                    pall = pp(f"pall{nm}{h}_{half}_{ci}")
                    for j in range(4):
                        n = half * 4 + j
                        kmsk = kms[n % 4]
                        blk = 64 * (n // 4)
                        nc.tensor.matmul(
                            pall[:, j * C:(j + 1) * C],
                            kmsk[blk:blk + 64, :],
                            srcq[blk:blk + 64, ts],
                            start=True, stop=True)
                    prod = wk.tile([128, 4 * C], bf16, tag="prod",
                                   name=f"prod{nm}{h}_{half}", bufs=2)
                    easl = ealls[h][:, half * 4 * C:(half + 1) * 4 * C]
                    nc.vector.scalar_tensor_tensor(prod[:], easl, 1.0,
                                                   pall[:], op0=Alu.min,
                                                   op1=Alu.mult)
                    prods.append(prod)
                red = pp(f"red{nm}{h}_{ci}")
                for n in range(NG):
                    nc.tensor.matmul(red[:, 0:C], idb[:],
                                     prods[n // 4][:, (n % 4) * C:
                                                   (n % 4 + 1) * C],
                                     start=(n == 0), stop=(n == NG - 1))
                if scale_col is not None:
                    nc.vector.scalar_tensor_tensor(dst, red[:, 0:C],
                                                   scale_col, mask_t[:],
                                                   op0=Alu.mult, op1=Alu.mult)
                else:
                    nc.vector.tensor_tensor(dst, red[:, 0:C], mask_t[:],
                                            op=Alu.mult)

            Hq0 = wk.tile([128, 4 * C], bf16, tag="Hq", name=f"Hq{ci}_0")
            Gq = wk.tile([128, 4 * C], bf16, tag="Gq", name=f"Gq{ci}")
            ax = dict(ts=ts, beta2=beta2, bCq=bCq, negWt=negWt,
                      qtT=qtT, kend=kend, Hq0=Hq0, Gq=Gq, ci=ci,
                      ealls=ealls, corr=corr, c64=c64, n64=n64)
            ax_store[ci] = ax

        def _a_eall(ci, h):
            ax = ax_store[ci]
            c64, n64 = ax['c64'], ax['n64']
            base = h * NG * C
            ea = wk.tile([128, NG * C], bf16, tag="eall",
                         name=f"eall{h}_{ci}", bufs=2)
            for half in range(2):
                bca = pp(f"bca{h}_{half}_{ci}")
                nc.tensor.matmul(bca[:],
                                 n64[:],
                                 s64c[:, base + half * 512:base + half * 512
                                      + 512],
                                 start=True, stop=False)
                for j in range(4):
                    n = half * 4 + j
                    nc.tensor.matmul(bca[:, j * C:(j + 1) * C],
                                     s64c[:, base + n * 128:base
                                          + (n + 1) * 128],
                                     c64[:], start=False, stop=(j == 3))
                nc.scalar.activation(
                    ea[:, half * 4 * C:(half + 1) * 4 * C], bca[:],
                    Act.Exp, scale=-1.0)
            ax['ealls'][h] = ea

        def _a_corr(ci, h, which):
            ax = ax_store[ci]
            if which == 'M':
                ax['corr'](h, kb[h], nmM, ax['beta2'][:, h:h + 1], "M",
                           ax['Hq0'][:, h * C:(h + 1) * C])
            else:
                ax['corr'](h, qb[h], mGt, None, "G",
                           ax['Gq'][:, h * C:(h + 1) * C])

        def sa_pieces(ci):
            ps = [lambda ci=ci: _a_prep(ci)]
            for h in range(4):
                ps.append(lambda ci=ci, h=h: _a_eall(ci, h))
                ps.append(lambda ci=ci, h=h: _a_corr(ci, h, 'M'))
                ps.append(lambda ci=ci, h=h: _a_corr(ci, h, 'G'))
            return ps

        # ---- work queue ----
        WQ = []
        wq_pos = [0]

        def pump(n):
            e = min(wq_pos[0] + n, len(WQ))
            while wq_pos[0] < e:
                WQ[wq_pos[0]]()
                wq_pos[0] += 1

        def drain_until(mark):
            while wq_pos[0] < mark:
                WQ[wq_pos[0]]()
                wq_pos[0] += 1

        def stage_b(ci, allow_pump=True):
            pmp = pump if allow_pump else (lambda n: None)
            ax = ax_store[ci]
            ts = ax['ts']
            beta2, bCq = ax['beta2'], ax['bCq']
            negWt, qtT, kend = ax['negWt'], ax['qtT'], ax['kend']
            Hq, Gq = ax['Hq0'], ax['Gq']

            xaq = pr.tile([128, 4 * C], fp32, tag="xaq", bufs=1,
                          name=f"xaq{ci}")
            xaccs = [xaq[:, h * C:(h + 1) * C] for h in range(4)]
            for h in range(4):
                nc.tensor.matmul(xaccs[h], vb[h][:, ts], idb[:],
                                 start=True, stop=False)
                nc.tensor.matmul(xaccs[h], negWt[h][:], Sb[h][:],
                                 start=False, stop=True)
            for lev in range(NLEV):
                pmp(3)
                last = (lev == NLEV - 1)
                xbq = wk.tile([128, 4 * C], bf16, tag="xbq",
                              name=f"xbq{ci}_{lev}")
                nc.scalar.copy(xbq[:], xaq[:])
                xaq = pr.tile([128, 4 * C], fp32, tag="xaq", bufs=1,
                              name=f"xaq{ci}_{lev}")
                xaccs = [xaq[:, h * C:(h + 1) * C] for h in range(4)]
                for h in range(4):
                    hs_ = slice(h * C, (h + 1) * C)
                    nc.tensor.matmul(xaccs[h], idb[:], xbq[:, hs_],
                                     start=True, stop=False)
                    nc.tensor.matmul(xaccs[h], Hq[:, hs_], xbq[:, hs_],
                                     start=False, stop=True)
                if not last:
                    htrq = q16(f"htr{ci}_{lev}")
                    for h in range(4):
                        nc.tensor.transpose(htrq[:, h * C:(h + 1) * C],
                                            Hq[:, h * C:(h + 1) * C],
                                            idb[:])
                    htsq = wk.tile([128, 4 * C], bf16, tag="htsq",
                                   name=f"htsq{ci}_{lev}", bufs=1)
                    nc.scalar.copy(htsq[:], htrq[:])
                    h2q = q32(f"h2q{ci}_{lev}")
                    for h in range(4):
                        hs_ = slice(h * C, (h + 1) * C)
                        nc.tensor.matmul(h2q[:, hs_], htsq[:, hs_],
                                         Hq[:, hs_], start=True, stop=True)
                    Hq = wk.tile([128, 4 * C], bf16, tag="Hq",
                                 name=f"Hq{ci}_{lev + 1}")
                    nc.scalar.copy(Hq[:], h2q[:])

            ubs = []
            for h in range(4):
                ub = wk.tile([128, C], bf16, tag=f"ub{h}", name=f"ub{h}_{ci}")
                nc.vector.tensor_scalar(ub[:], xaccs[h], beta2[:, h:h + 1],
                                        None, op0=Alu.mult)
                ubs.append(ub)
            otq = q32(f"otq{ci}")
            ktq = q16(f"ktq{ci}")
            for h in range(4):
                hs_ = slice(h * C, (h + 1) * C)
                nc.tensor.matmul(otq[:, hs_], Sb[h][:], qtT[h][:],
                                 start=True, stop=False)
                nc.tensor.matmul(otq[:, hs_], ubs[h][:], Gq[:, hs_],
                                 start=False, stop=True)
                nc.tensor.transpose(ktq[:, hs_], kend[h][:], idb[:])
            pmp(2)
            ktsq = wk.tile([128, 4 * C], bf16, tag="ktsq", name=f"ktsq{ci}", bufs=1)
            nc.scalar.copy(ktsq[:], ktq[:])
            suq = q32(f"suq{ci}")
            for h in range(4):
                hs_ = slice(h * C, (h + 1) * C)
                nc.tensor.matmul(suq[:, hs_], ktsq[:, hs_], ubs[h][:],
                                 start=True, stop=True)
                nc.vector.scalar_tensor_tensor(Sf[h][:], Sf[h][:],
                                               bCq[:, h:h + 1], suq[:, hs_],
                                               op0=Alu.mult, op1=Alu.add)
                nc.scalar.copy(Sb[h][:], Sf[h][:])
            sspq = None
            for h in range(4):
                hs_ = slice(h * C, (h + 1) * C)
                yf = yfall[h]
                nc.vector.tensor_tensor(yf[:, ts], gateb[h][:, ts],
                                        otq[:, hs_], op=Alu.mult)
                ysq = wk.tile([128, C], bf16, tag=f"ysq{h}", name=f"ysq{h}_{ci}")
                nc.gpsimd.tensor_tensor(ysq[:], yf[:, ts], yf[:, ts],
                                        op=Alu.mult)
                if h == 0:
                    sspq = pr.tile([128, 4 * C], fp32, tag="xaq", bufs=1,
                                   name=f"ssp{ci}")
                nc.tensor.matmul(sspq[0:4, 0:C], oh4t[:, 4 * h:4 * h + 4],
                                 ysq[:], start=(h == 0), stop=(h == 3))
                if h == 3:
                    nc.scalar.copy(sqs[:, ts], sspq[0:4, 0:C])
            pmp(3)

        # =================== EMISSION ===================
        # Segment A: half-0 projections + gates + norms, then chunk 0 prep.
        for m in range(4):
            proj_piece(0, m, 0)
        load_wset(1)
        for m in range(4):
            proj_piece(1, m, 0)
        silu_qk(0)               # Silu/Tanh table
        load_wset(2)
        for m in range(4):
            proj_piece(2, m, 0)
        load_wset(3)
        for m in range(4):
            proj_piece(3, m, 0)
        silu_vg(0)
        beta_piece()             # tanh shares the silu table
        fgate_piece()            # Exp/Ln table (stays for the rest)
        norm_piece(0, 0)
        norm_piece(0, 1)
        for p in sa_pieces(0):
            p()

        # Work queue: half-1 projections + chunk preps + output pieces.
        marks = {}
        for m in range(4):
            WQ.append(lambda m=m: (load_wset(0) if m == 0 else None,
                                   proj_piece(0, m, 1)))
        for p in sa_pieces(1):
            WQ.append(p)
        marks[1] = len(WQ)
        WQ.append(lambda: load_wset(1))
        for m in range(4):
            WQ.append(lambda m=m: proj_piece(1, m, 1))
        WQ.append(lambda: silu_qk(1))
        for p in sa_pieces(2):
            WQ.append(p)
        marks[2] = len(WQ)
        WQ.append(lambda: load_wset(2))
        for m in range(4):
            WQ.append(lambda m=m: proj_piece(2, m, 1))
        for p in sa_pieces(3):
            WQ.append(p)
        marks[3] = len(WQ)
        WQ.append(lambda: load_wset(3))
        for m in range(4):
            WQ.append(lambda m=m: proj_piece(3, m, 1))
        WQ.append(lambda: silu_vg(1))
        WQ.append(lambda: norm_piece(1, 0))
        WQ.append(lambda: norm_piece(1, 1))
        for p in sa_pieces(4):
            WQ.append(p)
        marks[4] = len(WQ)
        WQ.append(wo_piece)
        WQ.append(lambda: onorm_piece(0))
        for m in range(8):
            WQ.append(lambda m=m: om_piece(0, m))
        for p in sa_pieces(5):
            WQ.append(p)
        marks[5] = len(WQ)
        for m in range(8, 16):
            WQ.append(lambda m=m: om_piece(0, m))
        for p in sa_pieces(6):
            WQ.append(p)
        marks[6] = len(WQ)
        for p in sa_pieces(7):
            WQ.append(p)
        marks[7] = len(WQ)
        WQ.append(lambda: onorm_piece(1))
        for m in range(16):
            WQ.append(lambda m=m: om_piece(1, m))
        marks[8] = len(WQ)

        for ci in range(NCH):
            if ci > 0:
                drain_until(marks[ci])
            stage_b(ci, allow_pump=(ci < NCH - 1))
        drain_until(marks[8])
        if DBG:
            for m in range(4):
                dma(dbg[f'd_qb{m}'], qb[m][:])
                dma(dbg[f'd_kb{m}'], kb[m][:])
                dma(dbg[f'd_vb{m}'], vb[m][:])
                dma(dbg[f'd_gateb{m}'], gateb[m][:])
                dma(dbg[f'd_yf{m}'], yfall[m][:])
            dma(dbg['d_gna'], gna32[:])
            dma(dbg['d_bsg'], bsg[:])
            dma(dbg['d_sqs'], sqs[:])

    nc.compile()
    return nc


def _prep_inputs(inputs):
    f32 = np.float32
    hs = np.asarray(inputs['hidden_states'], f32)
    maps = []
    tri = np.tril(np.ones((C, C), f32))
    nmaskM = (-(1.0 - tri)).astype(BF)                      # -1 strictly upper
    maskG = (1.0 - tri + np.eye(C, dtype=f32)).astype(BF)   # +1 upper incl diag
    repl = np.zeros((NG, DK), f32)
    for n in range(NG):
        repl[n, n * GG:(n + 1) * GG] = 1.0
    sel8 = np.zeros((NG, NG * 128), f32)
    for n in range(NG):
        sel8[n, n * 128:(n + 1) * 128] = 1.0
    # per-head selectors on a 64-partition layout: rows 8h+g (hi) and
    # 32+8h+g (lo)
    s64 = np.zeros((64, 4 * NG * 128), f32)
    repl64 = np.zeros((64, 4 * DK), f32)
    for h in range(NH):
        s64[8 * h:8 * h + 8, h * 1024:(h + 1) * 1024] = sel8
        s64[32 + 8 * h:32 + 8 * h + 8, h * 1024:(h + 1) * 1024] = sel8
        repl64[8 * h:8 * h + 8, h * DK:(h + 1) * DK] = repl
        repl64[32 + 8 * h:32 + 8 * h + 8, h * DK:(h + 1) * DK] = repl
    oh8 = np.zeros((DK, 64), f32)
    for i in range(8):
        oh8[:, i * 8 + i] = 1.0
    oh4 = np.zeros((DK, 16), f32)
    for i in range(4):
        oh4[:, i * 4 + i] = 1.0
    oh4b = np.zeros((4, 4 * DK), f32)
    for i in range(4):
        oh4b[i, i * 128:(i + 1) * 128] = 1.0
    evodb = np.zeros((DK, 4 * C), f32)
    for cc in range(DK):
        evodb[cc, ((cc // GG) % 4) * C:(((cc // GG) % 4) + 1) * C] = 1.0
    ident = np.eye(128, dtype=f32)
    hTs = [np.ascontiguousarray(hs[b].T).astype(BF) for b in range(B)]
    for c in range(8):
        b, hg = c // 4, c % 4
        cols = slice(hg * NH * DK, (hg + 1) * NH * DK)
        gcols = slice(hg * NH * NG, (hg + 1) * NH * NG)
        hcols = slice(hg * NH, (hg + 1) * NH)
        Alog = np.asarray(inputs['A_log'], f32)[hcols]
        nega32 = np.repeat(np.exp(Alog), NG)[:, None]       # rows 8h+g
        dtb32 = np.asarray(inputs['dt_bias'], f32)[gcols].reshape(
            NH, NG).reshape(32)[:, None]
        m = {
            'hT': hTs[b],
            'wqkvg': np.ascontiguousarray(np.concatenate(
                [np.asarray(inputs['Wq'], f32)[:, cols],
                 np.asarray(inputs['Wk'], f32)[:, cols],
                 np.asarray(inputs['Wv'], f32)[:, cols],
                 np.asarray(inputs['Wg'], f32)[:, cols]], 1)).astype(BF),
            'wo': np.asarray(inputs['Wo'], f32)[cols, :].astype(BF),
            'wf1': np.asarray(inputs['Wf1'], f32).astype(BF),
            'wf2': np.asarray(inputs['Wf2'], f32)[:, gcols].astype(BF),
            'wb': np.asarray(inputs['Wb'], f32)[:, hcols].astype(BF),
            'cw': np.ascontiguousarray(np.concatenate(
                [np.asarray(inputs['conv_q'], f32)[cols],
                 np.asarray(inputs['conv_k'], f32)[cols],
                 np.asarray(inputs['conv_v'], f32)[cols]], 1)),
            'nega32': np.ascontiguousarray(nega32, f32),
            'dtb32': np.ascontiguousarray(dtb32, f32),
            'bgc': np.ascontiguousarray(
                np.asarray(inputs['bg'], f32)[cols].reshape(NH, DV).T),
            'normw': np.ascontiguousarray(
                np.asarray(inputs['norm_w'], f32)[:, None]),
            'repl64': repl64.astype(BF),
            's64f': s64.astype(BF),
            'sel8b': sel8.astype(BF),
            'oh4': oh4.astype(BF),
            'oh4b': oh4b.astype(BF),
            'evodb': evodb.astype(BF),
            'oh8': oh8.astype(BF),
            'sc8': np.array([[1.0 / SCALE ** 2]] * 4 + [[1.0]] * 4, f32),
            'eps8': np.array([[1e-6 / SCALE ** 2]] * 4 + [[1e-6]] * 4, f32),
            'nmaskM': nmaskM,
            'maskG': maskG,
            'idbf': ident.astype(BF),
        }
        maps.append(m)
    return maps


def kernel(**inputs):
    from concourse.bass_utils import run_bass_kernel_spmd
    if 'nc' not in _CACHE:
        _CACHE['nc'] = _build()
    nc = _CACHE['nc']
    maps = _prep_inputs(inputs)
    res = run_bass_kernel_spmd(nc, maps, list(range(8))).results
    out = np.zeros((B, T, D), np.float32)
    for c in range(8):
        out[c // 4] += res[c]['outT'].T.astype(np.float32)
    return out


# revision 32
# speedup vs baseline: 1.0183x; 1.0183x over previous
"""Grouped gated DeltaNet (KDA-style) on 8 TRN2 NeuronCores — v3.

Sharding: core c -> (batch b = c//4, head-group hg = c%4 of 4 heads).

v3 restructure vs v2:
- single software-pipelined emission stream: half-1 projections, the
  output projection, and DMA-out are drained into the chunk recurrence
  via a work queue (no sequential phase barriers)
- head-stacked gate path: one [32,T] gna tile, one cN scan per chunk,
  [64,C] hi/lo decay tile with per-head selector consts
- decay-difference build folded: stacked hi+lo contraction (1+4 matmuls
  per head-half instead of 2+8)
- conv in bf16 (DVE 2x mode); act-table switches confined to two silu
  batches (tanh-based sigmoid shares the silu table)
- weights streamed per half through a 2-deep ring (SBUF fit)
- per-half output projection pieces overlap the second half of the
  recurrence; DMA-out streams during compute

Self-contained: B=2, T=1024, D=2048, H=16, DK=DV=128 hardcoded.
"""
import sys
sys.path.insert(0, '/opt/trn_rl_repo')
import numpy as np
import ml_dtypes
from contextlib import ExitStack

B, T, D = 2, 1024, 2048
H, DK, DV, GG = 16, 128, 128, 16
NG = DK // GG          # 8 gate groups per head
NH = 4                 # heads per core
C = 128                # chunk length
NCH = T // C
NLEV = 2               # Neumann doubling levels (covers N^k, k < 2^NLEV)
SCALE = DK ** -0.5
EPS = 1e-5

BF = ml_dtypes.bfloat16
_CACHE = {}


def _build():
    import concourse.tile as tile
    from concourse import bacc, mybir

    fp32 = mybir.dt.float32
    bf16 = mybir.dt.bfloat16
    Alu = mybir.AluOpType
    Act = mybir.ActivationFunctionType

    nc = bacc.Bacc("TRN2", target_bir_lowering=False, debug=False, num_devices=8)
    dp = lambda n, sh, dt: nc.dram_tensor(n, sh, dt, kind="ExternalInput").ap()
    hT = dp("hT", [D, T], bf16)
    wqkvg = dp("wqkvg", [D, 4 * NH * DK], bf16)
    wo = dp("wo", [NH * DV, D], bf16)
    wf1 = dp("wf1", [D, DV], bf16)
    wf2 = dp("wf2", [DV, NH * NG], bf16)
    wb = dp("wb", [D, NH], bf16)
    cw = dp("cw", [NH * DK, 12], fp32)
    nega32 = dp("nega32", [32, 1], fp32)
    dtb32 = dp("dtb32", [32, 1], fp32)
    bgc = dp("bgc", [DV, NH], fp32)
    normw = dp("normw", [DV, 1], fp32)
    repl64 = dp("repl64", [64, 4 * DK], bf16)
    s64f = dp("s64f", [64, 4 * NG * C], bf16)
    oh4 = dp("oh4", [DK, 16], bf16)
    oh4b = dp("oh4b", [4, 4 * DK], bf16)
    evodb = dp("evodb", [DK, 4 * C], bf16)
    oh8 = dp("oh8", [DK, 64], bf16)
    sel8b = dp("sel8b", [8, 8 * 128], bf16)
    sc8 = dp("sc8", [8, 1], fp32)
    eps8 = dp("eps8", [8, 1], fp32)
    nmaskM = dp("nmaskM", [C, C], bf16)   # -1 strictly upper (s<t)
    maskG = dp("maskG", [C, C], bf16)     # +1 upper incl diag (s<=t)
    idbf = dp("idbf", [128, 128], bf16)
    outT = nc.dram_tensor("outT", [D, T], bf16, kind="ExternalOutput").ap()
    DBG = bool(__import__('os').environ.get('K3_DEBUG'))
    dbg = {}
    if DBG:
        for nm in ['d_qb', 'd_kb', 'd_vb', 'd_gateb', 'd_yf']:
            for m in range(4):
                dbg[f'{nm}{m}'] = nc.dram_tensor(
                    f'{nm}{m}', [128, T], bf16, kind="ExternalOutput").ap()
        dbg['d_gna'] = nc.dram_tensor('d_gna', [32, T], fp32,
                                      kind="ExternalOutput").ap()
        dbg['d_bsg'] = nc.dram_tensor('d_bsg', [4, T], bf16,
                                      kind="ExternalOutput").ap()
        dbg['d_sqs'] = nc.dram_tensor('d_sqs', [4, T], fp32,
                                      kind="ExternalOutput").ap()

    with tile.TileContext(nc) as tc, ExitStack() as ctx:
        pool = lambda name, bufs, space="SBUF": ctx.enter_context(
            tc.tile_pool(name=name, bufs=bufs, space=space))

        cons = pool("cons", 1)
        pers = pool("pers", 1)
        st = pool("st", 1)
        wk = pool("wk", 2)
        pr = pool("pr", 1, "PSUM")

        dma = nc.sync.dma_start

        adma = nc.scalar.dma_start   # second DMA queue (Act engine)

        # ---- conv weights (Act queue; needed ~25us in) ----
        cwt = []
        for m in range(4):
            t = cons.tile([128, 12], fp32, tag=f"cw{m}", name=f"cw{m}")
            adma(t[:], cw[m * 128:(m + 1) * 128, :])
            cwt.append(t)

        # single strided DMAs: (k p) x -> p k x gathers of the row-tiles
        wtiles = {}

        def load_wset(proj):
            ws = wk.tile([128, 16 * 512], bf16, tag="wset", name=f"ws{proj}",
                         bufs=2)
            for qt in range(4):
                nc.sync.dma_start(
                    ws[:, qt * 4 * 512:(qt + 1) * 4 * 512].rearrange(
                        "p (k c) -> p k c", k=4),
                    wqkvg[qt * 4 * 128:(qt + 1) * 4 * 128,
                          proj * 512:(proj + 1) * 512].rearrange(
                        "(k p) c -> p k c", k=4, p=128))
            wtiles[proj] = ws

        htab = [pers.tile([128, 16 * 512], bf16, tag=f"hth{hf}",
                          name=f"hth{hf}") for hf in range(2)]

        def load_ht(hf):
            for qt in range(4):
                nc.sync.dma_start(
                    htab[hf][:, qt * 4 * 512:(qt + 1) * 4 * 512].rearrange(
                        "p (k c) -> p k c", k=4),
                    hT[qt * 4 * 128:(qt + 1) * 4 * 128,
                       hf * 512:(hf + 1) * 512].rearrange(
                        "(k p) c -> p k c", k=4, p=128))

        def hts(k, hf):
            return htab[hf][:, k * 512:(k + 1) * 512]

        # critical stream on SP, interleaved by quarter for fast start
        for qt in range(4):
            nc.sync.dma_start(
                htab[0][:, qt * 2048:(qt + 1) * 2048].rearrange(
                    "p (k c) -> p k c", k=4),
                hT[qt * 512:(qt + 1) * 512, 0:512].rearrange(
                    "(k p) c -> p k c", k=4, p=128))
            ws0 = wtiles.setdefault(0, wk.tile([128, 16 * 512], bf16,
                                               tag="wset", name="ws0",
                                               bufs=2))
            nc.sync.dma_start(
                ws0[:, qt * 2048:(qt + 1) * 2048].rearrange(
                    "p (k c) -> p k c", k=4),
                wqkvg[qt * 512:(qt + 1) * 512, 0:512].rearrange(
                    "(k p) c -> p k c", k=4, p=128))
        load_wset(1)
        load_ht(1)

        # ---- remaining weights + consts on the Act queue ----
        wf1a = cons.tile([128, 16 * 128], bf16, tag="wf1a", name="wf1a")
        adma(wf1a[:].rearrange("p (k c) -> p k c", k=16),
             wf1[:].rearrange("(k p) c -> p k c", k=16, p=128))
        wf1t = [wf1a[:, k * 128:(k + 1) * 128] for k in range(16)]
        wba = cons.tile([128, 64], bf16, tag="wba", name="wba")
        adma(wba[:].rearrange("p (k c) -> p k c", k=16),
             wb[:].rearrange("(k p) c -> p k c", k=16, p=128))
        wbt = [wba[:, k * 4:(k + 1) * 4] for k in range(16)]

        def ctile(shape, dt, src, nm):
            t = cons.tile(shape, dt, tag=nm, name=nm)
            adma(t[:], src[:])
            return t
        wf2t = ctile([128, 32], bf16, wf2, "wf2t")
        negat = ctile([32, 1], fp32, nega32, "negat")
        dtbt = ctile([32, 1], fp32, dtb32, "dtbt")
        bgt = ctile([128, 4], fp32, bgc, "bgt")
        nwt = ctile([128, 1], fp32, normw, "nwt")
        idb = ctile([128, 128], bf16, idbf, "idb")
        r64t = ctile([64, 4 * 128], bf16, repl64, "r64t")
        s64c = ctile([64, 4 * NG * C], bf16, s64f, "s64c")
        oh4t = ctile([128, 16], bf16, oh4, "oh4t")
        oh4bt = ctile([4, 4 * 128], bf16, oh4b, "oh4bt")
        evt = ctile([128, 4 * C], bf16, evodb, "evt")
        oh8t = ctile([128, 64], bf16, oh8, "oh8t")
        s8b = ctile([8, 8 * 128], bf16, sel8b, "s8b")
        sc8t = ctile([8, 1], fp32, sc8, "sc8t")
        eps8t = ctile([8, 1], fp32, eps8, "eps8t")
        nmM = ctile([128, 128], bf16, nmaskM, "nmM")
        mGt = ctile([128, 128], bf16, maskG, "mGt")
        ones32 = cons.tile([32, C], fp32, tag="ones32", name="ones32")
        nc.vector.memset(ones32[:], 1.0)
        eps4 = cons.tile([4, 1], fp32, tag="eps4", name="eps4")
        nc.vector.memset(eps4[:], EPS)
        neg4c = cons.tile([128, 4 * C], bf16, tag="neg4c", name="neg4c")
        nc.vector.memset(neg4c[:], -1.0)

        # ---- persistent activations ----
        mk = lambda p, nm: [p.tile([128, T], bf16, tag=f"{nm}{m}",
                                   name=f"{nm}{m}") for m in range(4)]
        qb, kb, vb = mk(pers, "qb"), mk(pers, "kb"), mk(pers, "vb")
        gateb, yfall = mk(pers, "gateb"), mk(pers, "yfall")
        gna32 = cons.tile([32, T], fp32, tag="gna32", name="gna32")
        bsg = cons.tile([4, T], bf16, tag="bsg", name="bsg")
        sqs = cons.tile([4, T], fp32, tag="sqs", name="sqs")
        f1b = cons.tile([128, T], bf16, tag="f1b", name="f1b")

        # ---- PSUM rings: pp x4 (proj/bca/pall/red/ssq/out), xaq x1,
        #      q32a x2, q16b x1 ----
        def pp(nm):
            return pr.tile([128, 4 * C], fp32, tag="pp", bufs=4, name=nm)

        def q32(nm):
            return pr.tile([128, 4 * C], fp32, tag="q32a", bufs=2, name=nm)

        def q16(nm):
            return pr.tile([128, 4 * C], bf16, tag="q16b", bufs=1, name=nm)

        # ---- projection pieces ----
        accs = {}       # (proj, m) -> conv accumulator [128, 512]
        bnds = {}       # (proj, m) -> 3-col conv boundary
        gcs = {}        # m -> gate pre-silu copy

        def proj_piece(proj, m, half):
            """16 matmuls; conv projs: PSUM->xpad copy + 4 DVE taps into
            acc; gate proj: PSUM->SBUF copy. Silu deferred to a batch."""
            wt = wtiles[proj]
            ps = pp(f"prj{proj}{m}{half}")
            for k in range(16):
                nc.tensor.matmul(ps[:, 0:512],
                                 wt[:, k * 512 + m * 128:k * 512
                                    + (m + 1) * 128],
                                 hts(k, half), start=(k == 0), stop=(k == 15))
            if proj == 3:
                gc = wk.tile([128, 512], bf16, tag=f"ac1{m}", name=f"gc{m}",
                             bufs=1)
                nc.scalar.copy(gc[:], ps[:, 0:512])
                gcs[m] = gc
                return
            xpad = wk.tile([128, 515], bf16, tag="xpad", name="xpad", bufs=2)
            if half == 0:
                nc.vector.memset(xpad[:, 0:3], 0.0)
            else:
                nc.vector.tensor_copy(xpad[:, 0:3], bnds[(proj, m)][:])
            nc.scalar.copy(xpad[:, 3:515], ps[:, 0:512])
            if half == 0:
                bnd = wk.tile([128, 3], bf16, tag=f"bnd{proj}{m}",
                              name=f"bnd{proj}{m}", bufs=1)
                nc.vector.tensor_copy(bnd[:], xpad[:, 512:515])
                bnds[(proj, m)] = bnd
            cwm = cwt[m]
            s = proj * 4
            a = wk.tile([128, 512], bf16, tag=f"ac{proj % 2}{m}",
                        name=f"ac{proj}{m}", bufs=1)
            b2 = wk.tile([128, 512], bf16, tag="acw", name="acw", bufs=2)
            nc.vector.tensor_scalar(b2[:], xpad[:, 3:515],
                                    cwm[:, s + 3:s + 4], None, op0=Alu.mult)
            cur, nxt = b2, a
            for kk in (2, 1, 0):
                nc.vector.scalar_tensor_tensor(
                    nxt[:], xpad[:, kk:kk + 512], cwm[:, s + kk:s + kk + 1],
                    cur[:], op0=Alu.mult, op1=Alu.add)
                cur, nxt = nxt, cur
            accs[(proj, m)] = a      # 3 stt steps end in `a`

        def silu_qk(half):
            sl = slice(half * 512, (half + 1) * 512)
            for m in range(4):
                nc.scalar.activation(qb[m][:, sl], accs[(0, m)][:], Act.Silu)
            for m in range(4):
                nc.scalar.activation(kb[m][:, sl], accs[(1, m)][:], Act.Silu)

        def silu_vg(half):
            sl = slice(half * 512, (half + 1) * 512)
            for m in range(4):
                nc.scalar.activation(vb[m][:, sl], accs[(2, m)][:], Act.Silu)
            for m in range(4):
                nc.scalar.activation(gateb[m][:, sl], gcs[m][:], Act.Silu,
                                     bias=bgt[:, m:m + 1])

        def beta_piece():
            for half in range(2):
                bps = pp(f"bps{half}")
                for k in range(16):
                    nc.tensor.matmul(bps[0:4, 0:512], wbt[k],
                                     hts(k, half),
                                     start=(k == 0), stop=(k == 15))
                # sigmoid(x) = 0.5 + 0.5*tanh(x/2) (shares the silu table)
                tb = wk.tile([4, 512], bf16, tag="tb", name=f"tb{half}")
                nc.scalar.activation(tb[:], bps[0:4, 0:512], Act.Tanh,
                                     scale=0.5)
                nc.vector.tensor_scalar(bsg[:, half * 512:(half + 1) * 512],
                                        tb[:], 0.5, 0.5, op0=Alu.mult,
                                        op1=Alu.add)

        def fgate_piece():
            """f1 projection + grouped softplus gate, both halves (Exp/Ln)."""
            for half in range(2):
                sl = slice(half * 512, (half + 1) * 512)
                ps = pp(f"f1p{half}")
                for k in range(16):
                    nc.tensor.matmul(ps[:, 0:512], wf1t[k], hts(k, half),
                                     start=(k == 0), stop=(k == 15))
                nc.scalar.copy(f1b[:, sl], ps[:, 0:512])
            for half in range(2):
                sl = slice(half * 512, (half + 1) * 512)
                gp = pp(f"gp{half}")
                nc.tensor.matmul(gp[0:32, 0:512], wf2t[:, 0:32], f1b[:, sl],
                                 start=True, stop=True)
                spe = wk.tile([32, 512], fp32, tag="spe", name=f"spe{half}", bufs=1)
                nc.scalar.activation(spe[:], gp[0:32, 0:512], Act.Exp,
                                     bias=dtbt[:, 0:1])
                sp = wk.tile([32, 512], fp32, tag="spx", name=f"sp{half}", bufs=1)
                nc.scalar.activation(sp[:], spe[:], Act.Ln,
                                     bias=ones32[:, 0:1])
                nc.vector.tensor_scalar(gna32[:, sl], sp[:], negat[:, 0:1],
                                        None, op0=Alu.mult)

        def norm_piece(half, grp):
            """l2 normalizers for pairs grp*4..grp*4+3 (q: grp 0, k: grp 1)."""
            sl = slice(half * 512, (half + 1) * 512)
            rows = slice(grp * 4, grp * 4 + 4)
            ssq = pp(f"ssq{half}{grp}")
            for i in range(4):
                p = grp * 4 + i
                src = qb[p][:, sl] if p < 4 else kb[p - 4][:, sl]
                sq = wk.tile([128, 512], bf16, tag="sq", name=f"sq{p}{half}", bufs=1)
                nc.vector.tensor_tensor(sq[:], src, src, op=Alu.mult)
                nc.tensor.matmul(ssq[0:8, 0:512], oh8t[:, p * 8:p * 8 + 8],
                                 sq[:], start=(i == 0), stop=(i == 3))
            nrm = wk.tile([8, 512], fp32, tag="nrm", name=f"nrm{half}{grp}", bufs=1)
            nc.scalar.activation(nrm[:], ssq[0:8, 0:512], Act.Ln,
                                 scale=sc8t[:, 0:1], bias=eps8t[:, 0:1])
            recb = wk.tile([8, 512], bf16, tag="recb", name=f"rec{half}{grp}")
            nc.scalar.activation(recb[:], nrm[:], Act.Exp, scale=-0.5)
            for i in range(4):
                p = grp * 4 + i
                dst = qb[p][:, sl] if p < 4 else kb[p - 4][:, sl]
                nb = pp(f"nb{p}{half}")
                nc.tensor.matmul(nb[:, 0:512], s8b[:, p * 128:(p + 1) * 128],
                                 recb[:], start=True, stop=True)
                nc.vector.tensor_tensor(dst, dst, nb[:, 0:512], op=Alu.mult)

        # ---- output projection pieces ----
        wot = {}

        def wo_load(mh):
            wot.clear()
            for k in range(4):
                t = wk.tile([128, 1024], bf16, tag=f"wo{k}", name=f"wo{k}{mh}",
                            bufs=1)
                dma(t[:], wo[k * 128:(k + 1) * 128,
                             mh * 1024:(mh + 1) * 1024])
                wot[k] = t

        def onorm_piece(c0, c1):
            w = c1 - c0
            sl = slice(c0, c1)
            nrm4 = wk.tile([4, 512], fp32, tag="nrm4", name=f"nrm4{c0}", bufs=1)
            nc.scalar.activation(nrm4[:, 0:w], sqs[:, sl], Act.Ln,
                                 scale=1.0 / DV, bias=eps4[:, 0:1])
            rst4 = wk.tile([4, 512], bf16, tag="rst4", name=f"rst4{c0}", bufs=1)
            nc.scalar.activation(rst4[:, 0:w], nrm4[:, 0:w], Act.Exp,
                                 scale=-0.5)
            for h in range(4):
                rbc = pp(f"rbc{h}_{c0}")
                nc.tensor.matmul(rbc[:, 0:w], oh4bt[:, h * 128:(h + 1) * 128],
                                 rst4[:, 0:w], start=True, stop=True)
                nc.vector.scalar_tensor_tensor(yfall[h][:, sl], yfall[h][:, sl],
                                               nwt[:, 0:1], rbc[:, 0:w],
                                               op0=Alu.mult, op1=Alu.mult)

        def om_piece(m, c0, c1):
            w = c1 - c0
            sl = slice(c0, c1)
            ps = pp(f"ops{m}_{c0}")
            mm = m % 8
            for k in range(4):
                nc.tensor.matmul(ps[:, 0:w],
                                 wot[k][:, mm * 128:(mm + 1) * 128],
                                 yfall[k][:, sl], start=(k == 0), stop=(k == 3))
            osb = wk.tile([128, 512], bf16, tag="osb", name=f"osb{m}{c0}",
                          bufs=2)
            if m % 2 == 0:
                nc.vector.tensor_copy(osb[:, 0:w], ps[:, 0:w])
            else:
                nc.scalar.copy(osb[:, 0:w], ps[:, 0:w])
            dma(outT[m * 128:(m + 1) * 128, sl], osb[:, 0:w])

        # ---- recurrence (stage_a / stage_b) ----
        Sf = [st.tile([128, 128], fp32, tag=f"Sf{h}", name=f"Sf{h}")
              for h in range(4)]
        Sb = [st.tile([128, 128], bf16, tag=f"Sb{h}", name=f"Sb{h}")
              for h in range(4)]
        for h in range(4):
            nc.vector.memset(Sf[h][:], 0.0)
            nc.vector.memset(Sb[h][:], 0.0)

        ax_store = {}

        def _a_prep(ci):
            ts = slice(ci * C, (ci + 1) * C)
            prep = pr.tile([128, 4], bf16, tag="q32a", bufs=2,
                           name=f"prep{ci}")
            nc.tensor.transpose(prep[:], bsg[:, ts], idb[0:4, 0:4])
            beta2 = wk.tile([128, 4], fp32, tag="beta2", name=f"beta2_{ci}")
            nc.scalar.copy(beta2[:], prep[:])
            cN32 = wk.tile([32, C], fp32, tag="cN32", name=f"cN32_{ci}")
            nc.vector.tensor_tensor_scan(cN32[:], ones32[:], gna32[:, ts],
                                         0.0, op0=Alu.mult, op1=Alu.add)
            c64 = wk.tile([64, C], bf16, tag="c64", name=f"c64_{ci}")
            nc.scalar.copy(c64[0:32, :], cN32[:])
            nc.vector.tensor_tensor(c64[32:64, :], cN32[:], c64[0:32, :],
                                    op=Alu.subtract)
            n64 = wk.tile([64, C], bf16, tag="n64", name=f"n64_{ci}")
            nc.gpsimd.tensor_tensor(n64[:], c64[:], neg4c[0:64, 0:C],
                                    op=Alu.mult)

            # channel decay expansion, all heads in one quad
            cfq = q32(f"cfq{ci}")
            for h in range(4):
                hs_ = slice(h * C, (h + 1) * C)
                nc.tensor.matmul(cfq[:, hs_], r64t[:, h * 128:(h + 1) * 128],
                                 c64[:], start=True, stop=True)
            nclq = wk.tile([128, 4], fp32, tag="nclq", name=f"nclq{ci}")
            for h in range(4):
                nc.vector.tensor_scalar(nclq[:, h:h + 1],
                                        cfq[:, h * C + C - 1:h * C + C],
                                        -1.0, None, op0=Alu.mult)
            bfq = wk.tile([128, 4 * C], bf16, tag="bfq", name=f"bfq{ci}", bufs=1)
            nc.scalar.activation(bfq[:], cfq[:], Act.Exp, scale=-1.0)
            kfq = wk.tile([128, 4 * C], bf16, tag="kfq", name=f"kfq{ci}", bufs=1)
            for h in range(4):
                hs_ = slice(h * C, (h + 1) * C)
                nc.scalar.activation(kfq[:, hs_], cfq[:, hs_], Act.Exp,
                                     bias=nclq[:, h:h + 1])
            bCq = wk.tile([128, 4], fp32, tag="bCq", name=f"bCq{ci}")
            nc.scalar.activation(bCq[:], nclq[:], Act.Exp)
            nbfq = wk.tile([128, 4 * C], bf16, tag="nbfq", name=f"nbfq{ci}", bufs=1)
            nc.gpsimd.tensor_tensor(nbfq[:], bfq[:], neg4c[:], op=Alu.mult)

            # decayed k/q streams (Pool)
            negWt, qtT, kend = [], [], []
            for h in range(4):
                hs_ = slice(h * C, (h + 1) * C)
                nw = wk.tile([128, C], bf16, tag=f"negWt{h}",
                             name=f"negWt{h}_{ci}")
                nc.gpsimd.tensor_tensor(nw[:], kb[h][:, ts], nbfq[:, hs_],
                                        op=Alu.mult)
                qt = wk.tile([128, C], bf16, tag=f"qtT{h}", name=f"qtT{h}_{ci}")
                nc.gpsimd.tensor_tensor(qt[:], qb[h][:, ts], bfq[:, hs_],
                                        op=Alu.mult)
                ke = wk.tile([128, C], bf16, tag=f"kend{h}",
                             name=f"kend{h}_{ci}")
                nc.gpsimd.tensor_tensor(ke[:], kb[h][:, ts], kfq[:, hs_],
                                        op=Alu.mult)
                negWt.append(nw); qtT.append(qt); kend.append(ke)

            ealls = [None] * 4

            def corr(h, srcq, mask_t, scale_col, nm, dst):
                kms = []
                for j in range(4):
                    km = wk.tile([128, C], bf16, tag="km",
                                 name=f"km{j}_{h}_{nm}_{ci}", bufs=4)
                    nc.gpsimd.tensor_tensor(km[:], kb[h][:, ts],
                                            evt[:, j * C:(j + 1) * C],
                                            op=Alu.mult)
                    kms.append(km)
                prods = []
                for half in range(2):
                    pall = pp(f"pall{nm}{h}_{half}_{ci}")
                    for j in range(4):
                        n = half * 4 + j
                        kmsk = kms[n % 4]
                        blk = 64 * (n // 4)
                        nc.tensor.matmul(
                            pall[:, j * C:(j + 1) * C],
                            kmsk[blk:blk + 64, :],
                            srcq[blk:blk + 64, ts],
                            start=True, stop=True)
                    prod = wk.tile([128, 4 * C], bf16, tag="prod",
                                   name=f"prod{nm}{h}_{half}", bufs=2)
                    easl = ealls[h][:, half * 4 * C:(half + 1) * 4 * C]
                    nc.vector.scalar_tensor_tensor(prod[:], easl, 1.0,
                                                   pall[:], op0=Alu.min,
                                                   op1=Alu.mult)
                    prods.append(prod)
                red = pp(f"red{nm}{h}_{ci}")
                for n in range(NG):
                    nc.tensor.matmul(red[:, 0:C], idb[:],
                                     prods[n // 4][:, (n % 4) * C:
                                                   (n % 4 + 1) * C],
                                     start=(n == 0), stop=(n == NG - 1))
                if scale_col is not None:
                    nc.vector.scalar_tensor_tensor(dst, red[:, 0:C],
                                                   scale_col, mask_t[:],
                                                   op0=Alu.mult, op1=Alu.mult)
                else:
                    nc.vector.tensor_tensor(dst, red[:, 0:C], mask_t[:],
                                            op=Alu.mult)

            Hq0 = wk.tile([128, 4 * C], bf16, tag="Hq", name=f"Hq{ci}_0")
            Gq = wk.tile([128, 4 * C], bf16, tag="Gq", name=f"Gq{ci}")
            ax = dict(ts=ts, beta2=beta2, bCq=bCq, negWt=negWt,
                      qtT=qtT, kend=kend, Hq0=Hq0, Gq=Gq, ci=ci,
                      ealls=ealls, corr=corr, c64=c64, n64=n64)
            ax_store[ci] = ax

        def _a_eall(ci, h):
            ax = ax_store[ci]
            c64, n64 = ax['c64'], ax['n64']
            base = h * NG * C
            ea = wk.tile([128, NG * C], bf16, tag="eall",
                         name=f"eall{h}_{ci}", bufs=2)
            for half in range(2):
                bca = pp(f"bca{h}_{half}_{ci}")
                nc.tensor.matmul(bca[:],
                                 n64[:],
                                 s64c[:, base + half * 512:base + half * 512
                                      + 512],
                                 start=True, stop=False)
                for j in range(4):
                    n = half * 4 + j
                    nc.tensor.matmul(bca[:, j * C:(j + 1) * C],
                                     s64c[:, base + n * 128:base
                                          + (n + 1) * 128],
                                     c64[:], start=False, stop=(j == 3))
                nc.scalar.activation(
                    ea[:, half * 4 * C:(half + 1) * 4 * C], bca[:],
                    Act.Exp, scale=-1.0)
            ax['ealls'][h] = ea

        def _a_corr(ci, h, which):
            ax = ax_store[ci]
            if which == 'M':
                ax['corr'](h, kb[h], nmM, ax['beta2'][:, h:h + 1], "M",
                           ax['Hq0'][:, h * C:(h + 1) * C])
            else:
                ax['corr'](h, qb[h], mGt, None, "G",
                           ax['Gq'][:, h * C:(h + 1) * C])

        def sa_pieces(ci):
            ps = [lambda ci=ci: _a_prep(ci)]
            for h in range(4):
                ps.append(lambda ci=ci, h=h: _a_eall(ci, h))
                ps.append(lambda ci=ci, h=h: _a_corr(ci, h, 'M'))
                ps.append(lambda ci=ci, h=h: _a_corr(ci, h, 'G'))
            return ps

        # ---- work queue ----
        WQ = []
        wq_pos = [0]
        PUMP_CAP = [None]

        def pump(n):
            e = min(wq_pos[0] + n, len(WQ))
            if PUMP_CAP[0] is not None:
                e = min(e, PUMP_CAP[0])
            while wq_pos[0] < e:
                WQ[wq_pos[0]]()
                wq_pos[0] += 1

        def drain_until(mark):
            while wq_pos[0] < mark:
                WQ[wq_pos[0]]()
                wq_pos[0] += 1

        def stage_b(ci):
            pmp = pump
            ax = ax_store[ci]
            ts = ax['ts']
            beta2, bCq = ax['beta2'], ax['bCq']
            negWt, qtT, kend = ax['negWt'], ax['qtT'], ax['kend']
            Hq, Gq = ax['Hq0'], ax['Gq']

            xaq = pr.tile([128, 4 * C], fp32, tag="xaq", bufs=1,
                          name=f"xaq{ci}")
            xaccs = [xaq[:, h * C:(h + 1) * C] for h in range(4)]
            for h in range(4):
                nc.tensor.matmul(xaccs[h], vb[h][:, ts], idb[:],
                                 start=True, stop=False)
                nc.tensor.matmul(xaccs[h], negWt[h][:], Sb[h][:],
                                 start=False, stop=True)
            for lev in range(NLEV):
                pmp(3)
                last = (lev == NLEV - 1)
                xbq = wk.tile([128, 4 * C], bf16, tag="xbq",
                              name=f"xbq{ci}_{lev}", bufs=1)
                nc.scalar.copy(xbq[:], xaq[:])
                xaq = pr.tile([128, 4 * C], fp32, tag="xaq", bufs=1,
                              name=f"xaq{ci}_{lev}")
                xaccs = [xaq[:, h * C:(h + 1) * C] for h in range(4)]
                for h in range(4):
                    hs_ = slice(h * C, (h + 1) * C)
                    nc.tensor.matmul(xaccs[h], idb[:], xbq[:, hs_],
                                     start=True, stop=False)
                    nc.tensor.matmul(xaccs[h], Hq[:, hs_], xbq[:, hs_],
                                     start=False, stop=True)
                if not last:
                    htrq = q16(f"htr{ci}_{lev}")
                    for h in range(4):
                        nc.tensor.transpose(htrq[:, h * C:(h + 1) * C],
                                            Hq[:, h * C:(h + 1) * C],
                                            idb[:])
                    htsq = wk.tile([128, 4 * C], bf16, tag="htsq",
                                   name=f"htsq{ci}_{lev}", bufs=1)
                    nc.scalar.copy(htsq[:], htrq[:])
                    h2q = q32(f"h2q{ci}_{lev}")
                    for h in range(4):
                        hs_ = slice(h * C, (h + 1) * C)
                        nc.tensor.matmul(h2q[:, hs_], htsq[:, hs_],
                                         Hq[:, hs_], start=True, stop=True)
                    Hq = wk.tile([128, 4 * C], bf16, tag="Hq",
                                 name=f"Hq{ci}_{lev + 1}")
                    nc.scalar.copy(Hq[:], h2q[:])

            ubs = []
            for h in range(4):
                ub = wk.tile([128, C], bf16, tag=f"ub{h}", name=f"ub{h}_{ci}")
                nc.vector.tensor_scalar(ub[:], xaccs[h], beta2[:, h:h + 1],
                                        None, op0=Alu.mult)
                ubs.append(ub)
            otq = q32(f"otq{ci}")
            ktq = q16(f"ktq{ci}")
            for h in range(4):
                hs_ = slice(h * C, (h + 1) * C)
                nc.tensor.matmul(otq[:, hs_], Sb[h][:], qtT[h][:],
                                 start=True, stop=False)
                nc.tensor.matmul(otq[:, hs_], ubs[h][:], Gq[:, hs_],
                                 start=False, stop=True)
                nc.tensor.transpose(ktq[:, hs_], kend[h][:], idb[:])
            pmp(2)
            ktsq = wk.tile([128, 4 * C], bf16, tag="ktsq", name=f"ktsq{ci}", bufs=1)
            nc.scalar.copy(ktsq[:], ktq[:])
            suq = q32(f"suq{ci}")
            for h in range(4):
                hs_ = slice(h * C, (h + 1) * C)
                nc.tensor.matmul(suq[:, hs_], ktsq[:, hs_], ubs[h][:],
                                 start=True, stop=True)
                nc.vector.scalar_tensor_tensor(Sf[h][:], Sf[h][:],
                                               bCq[:, h:h + 1], suq[:, hs_],
                                               op0=Alu.mult, op1=Alu.add)
                nc.scalar.copy(Sb[h][:], Sf[h][:])
            sspq = None
            for h in range(4):
                hs_ = slice(h * C, (h + 1) * C)
                yf = yfall[h]
                nc.vector.tensor_tensor(yf[:, ts], gateb[h][:, ts],
                                        otq[:, hs_], op=Alu.mult)
                ysq = wk.tile([128, C], bf16, tag=f"ysq{h}", name=f"ysq{h}_{ci}")
                nc.gpsimd.tensor_tensor(ysq[:], yf[:, ts], yf[:, ts],
                                        op=Alu.mult)
                if h == 0:
                    sspq = pr.tile([128, 4 * C], fp32, tag="xaq", bufs=1,
                                   name=f"ssp{ci}")
                nc.tensor.matmul(sspq[0:4, 0:C], oh4t[:, 4 * h:4 * h + 4],
                                 ysq[:], start=(h == 0), stop=(h == 3))
                if h == 3:
                    nc.scalar.copy(sqs[:, ts], sspq[0:4, 0:C])
            pmp(3)

        # =================== EMISSION ===================
        # Segment A: half-0 projections + gates + norms, then chunk 0 prep.
        for m in range(4):
            proj_piece(0, m, 0)
        load_wset(1)
        for m in range(4):
            proj_piece(1, m, 0)
        silu_qk(0)               # Silu/Tanh table
        load_wset(2)
        for m in range(4):
            proj_piece(2, m, 0)
        load_wset(3)
        for m in range(4):
            proj_piece(3, m, 0)
        silu_vg(0)
        beta_piece()             # tanh shares the silu table
        fgate_piece()            # Exp/Ln table (stays for the rest)
        norm_piece(0, 0)
        norm_piece(0, 1)
        for p in sa_pieces(0):
            p()

        # Work queue: half-1 projections + chunk preps + output pieces.
        marks = {}
        for m in range(4):
            WQ.append(lambda m=m: (load_wset(0) if m == 0 else None,
                                   proj_piece(0, m, 1)))
        for p in sa_pieces(1):
            WQ.append(p)
        marks[1] = len(WQ)
        WQ.append(lambda: load_wset(1))
        for m in range(4):
            WQ.append(lambda m=m: proj_piece(1, m, 1))
        WQ.append(lambda: silu_qk(1))
        for p in sa_pieces(2):
            WQ.append(p)
        marks[2] = len(WQ)
        WQ.append(lambda: load_wset(2))
        for m in range(4):
            WQ.append(lambda m=m: proj_piece(2, m, 1))
        for p in sa_pieces(3):
            WQ.append(p)
        marks[3] = len(WQ)
        WQ.append(lambda: load_wset(3))
        for m in range(4):
            WQ.append(lambda m=m: proj_piece(3, m, 1))
        WQ.append(lambda: silu_vg(1))
        WQ.append(lambda: norm_piece(1, 0))
        WQ.append(lambda: norm_piece(1, 1))
        for p in sa_pieces(4):
            WQ.append(p)
        marks[4] = len(WQ)
        WQ.append(lambda: wo_load(0))
        WQ.append(lambda: onorm_piece(0, 512))
        for m in range(8):
            WQ.append(lambda m=m: om_piece(m, 0, 512))
        for p in sa_pieces(5):
            WQ.append(p)
        marks[5] = len(WQ)
        WQ.append(lambda: wo_load(1))
        for m in range(8, 16):
            WQ.append(lambda m=m: om_piece(m, 0, 512))
        for p in sa_pieces(6):
            WQ.append(p)
        marks[6] = len(WQ)
        for p in sa_pieces(7):
            WQ.append(p)
        marks[7] = len(WQ)
        WQ.append(lambda: wo_load(0))
        WQ.append(lambda: onorm_piece(512, 896))
        for m in range(8):
            WQ.append(lambda m=m: om_piece(m, 512, 896))
        WQ.append(lambda: wo_load(1))
        for m in range(8, 16):
            WQ.append(lambda m=m: om_piece(m, 512, 896))
        PUMP_CAP[0] = marks['tail'] = len(WQ)
        WQ.append(lambda: wo_load(0))
        WQ.append(lambda: onorm_piece(896, 1024))
        for m in range(8):
            WQ.append(lambda m=m: om_piece(m, 896, 1024))
        WQ.append(lambda: wo_load(1))
        for m in range(8, 16):
            WQ.append(lambda m=m: om_piece(m, 896, 1024))
        marks[8] = len(WQ)

        for ci in range(NCH):
            if ci > 0:
                drain_until(marks[ci])
            stage_b(ci)
        PUMP_CAP[0] = None
        drain_until(marks[8])
        if DBG:
            for m in range(4):
                dma(dbg[f'd_qb{m}'], qb[m][:])
                dma(dbg[f'd_kb{m}'], kb[m][:])
                dma(dbg[f'd_vb{m}'], vb[m][:])
                dma(dbg[f'd_gateb{m}'], gateb[m][:])
                dma(dbg[f'd_yf{m}'], yfall[m][:])
            dma(dbg['d_gna'], gna32[:])
            dma(dbg['d_bsg'], bsg[:])
            dma(dbg['d_sqs'], sqs[:])

    nc.compile()
    return nc


def _prep_inputs(inputs):
    f32 = np.float32
    hs = np.asarray(inputs['hidden_states'], f32)
    maps = []
    tri = np.tril(np.ones((C, C), f32))
    nmaskM = (-(1.0 - tri)).astype(BF)                      # -1 strictly upper
    maskG = (1.0 - tri + np.eye(C, dtype=f32)).astype(BF)   # +1 upper incl diag
    repl = np.zeros((NG, DK), f32)
    for n in range(NG):
        repl[n, n * GG:(n + 1) * GG] = 1.0
    sel8 = np.zeros((NG, NG * 128), f32)
    for n in range(NG):
        sel8[n, n * 128:(n + 1) * 128] = 1.0
    # per-head selectors on a 64-partition layout: rows 8h+g (hi) and
    # 32+8h+g (lo)
    s64 = np.zeros((64, 4 * NG * 128), f32)
    repl64 = np.zeros((64, 4 * DK), f32)
    for h in range(NH):
        s64[8 * h:8 * h + 8, h * 1024:(h + 1) * 1024] = sel8
        s64[32 + 8 * h:32 + 8 * h + 8, h * 1024:(h + 1) * 1024] = sel8
        repl64[8 * h:8 * h + 8, h * DK:(h + 1) * DK] = repl
        repl64[32 + 8 * h:32 + 8 * h + 8, h * DK:(h + 1) * DK] = repl
    oh8 = np.zeros((DK, 64), f32)
    for i in range(8):
        oh8[:, i * 8 + i] = 1.0
    oh4 = np.zeros((DK, 16), f32)
    for i in range(4):
        oh4[:, i * 4 + i] = 1.0
    oh4b = np.zeros((4, 4 * DK), f32)
    for i in range(4):
        oh4b[i, i * 128:(i + 1) * 128] = 1.0
    evodb = np.zeros((DK, 4 * C), f32)
    for cc in range(DK):
        evodb[cc, ((cc // GG) % 4) * C:(((cc // GG) % 4) + 1) * C] = 1.0
    ident = np.eye(128, dtype=f32)
    hTs = [np.ascontiguousarray(hs[b].T).astype(BF) for b in range(B)]
    for c in range(8):
        b, hg = c // 4, c % 4
        cols = slice(hg * NH * DK, (hg + 1) * NH * DK)
        gcols = slice(hg * NH * NG, (hg + 1) * NH * NG)
        hcols = slice(hg * NH, (hg + 1) * NH)
        Alog = np.asarray(inputs['A_log'], f32)[hcols]
        nega32 = np.repeat(np.exp(Alog), NG)[:, None]       # rows 8h+g
        dtb32 = np.asarray(inputs['dt_bias'], f32)[gcols].reshape(
            NH, NG).reshape(32)[:, None]
        m = {
            'hT': hTs[b],
            'wqkvg': np.ascontiguousarray(np.concatenate(
                [np.asarray(inputs['Wq'], f32)[:, cols],
                 np.asarray(inputs['Wk'], f32)[:, cols],
                 np.asarray(inputs['Wv'], f32)[:, cols],
                 np.asarray(inputs['Wg'], f32)[:, cols]], 1)).astype(BF),
            'wo': np.asarray(inputs['Wo'], f32)[cols, :].astype(BF),
            'wf1': np.asarray(inputs['Wf1'], f32).astype(BF),
            'wf2': np.asarray(inputs['Wf2'], f32)[:, gcols].astype(BF),
            'wb': np.asarray(inputs['Wb'], f32)[:, hcols].astype(BF),
            'cw': np.ascontiguousarray(np.concatenate(
                [np.asarray(inputs['conv_q'], f32)[cols],
                 np.asarray(inputs['conv_k'], f32)[cols],
                 np.asarray(inputs['conv_v'], f32)[cols]], 1)),
            'nega32': np.ascontiguousarray(nega32, f32),
            'dtb32': np.ascontiguousarray(dtb32, f32),
            'bgc': np.ascontiguousarray(
                np.asarray(inputs['bg'], f32)[cols].reshape(NH, DV).T),
            'normw': np.ascontiguousarray(
                np.asarray(inputs['norm_w'], f32)[:, None]),
            'repl64': repl64.astype(BF),
            's64f': s64.astype(BF),
            'sel8b': sel8.astype(BF),
            'oh4': oh4.astype(BF),
            'oh4b': oh4b.astype(BF),
            'evodb': evodb.astype(BF),
            'oh8': oh8.astype(BF),
            'sc8': np.array([[1.0 / SCALE ** 2]] * 4 + [[1.0]] * 4, f32),
            'eps8': np.array([[1e-6 / SCALE ** 2]] * 4 + [[1e-6]] * 4, f32),
            'nmaskM': nmaskM,
            'maskG': maskG,
            'idbf': ident.astype(BF),
        }
        maps.append(m)
    return maps


def kernel(**inputs):
    from concourse.bass_utils import run_bass_kernel_spmd
    if 'nc' not in _CACHE:
        _CACHE['nc'] = _build()
    nc = _CACHE['nc']
    maps = _prep_inputs(inputs)
    res = run_bass_kernel_spmd(nc, maps, list(range(8))).results
    out = np.zeros((B, T, D), np.float32)
    for c in range(8):
        out[c // 4] += np.asarray(res[c]['outT'], np.float32).T
    return out


# revision 33
# speedup vs baseline: 1.0459x; 1.0270x over previous
"""Grouped gated DeltaNet (KDA-style) on 8 TRN2 NeuronCores — v3.

Sharding: core c -> (batch b = c//4, head-group hg = c%4 of 4 heads).

v3 restructure vs v2:
- single software-pipelined emission stream: half-1 projections, the
  output projection, and DMA-out are drained into the chunk recurrence
  via a work queue (no sequential phase barriers)
- head-stacked gate path: one [32,T] gna tile, one cN scan per chunk,
  [64,C] hi/lo decay tile with per-head selector consts
- decay-difference build folded: stacked hi+lo contraction (1+4 matmuls
  per head-half instead of 2+8)
- conv in bf16 (DVE 2x mode); act-table switches confined to two silu
  batches (tanh-based sigmoid shares the silu table)
- weights streamed per half through a 2-deep ring (SBUF fit)
- per-half output projection pieces overlap the second half of the
  recurrence; DMA-out streams during compute

Self-contained: B=2, T=1024, D=2048, H=16, DK=DV=128 hardcoded.
"""
import sys
sys.path.insert(0, '/opt/trn_rl_repo')
import numpy as np
import ml_dtypes
from contextlib import ExitStack

B, T, D = 2, 1024, 2048
H, DK, DV, GG = 16, 128, 128, 16
NG = DK // GG          # 8 gate groups per head
NH = 4                 # heads per core
C = 128                # chunk length
NCH = T // C
NLEV = 2               # Neumann doubling levels (covers N^k, k < 2^NLEV)
SCALE = DK ** -0.5
EPS = 1e-5

BF = ml_dtypes.bfloat16
_CACHE = {}


def _build():
    import concourse.tile as tile
    from concourse import bacc, mybir

    fp32 = mybir.dt.float32
    bf16 = mybir.dt.bfloat16
    Alu = mybir.AluOpType
    Act = mybir.ActivationFunctionType

    nc = bacc.Bacc("TRN2", target_bir_lowering=False, debug=False, num_devices=8)
    dp = lambda n, sh, dt: nc.dram_tensor(n, sh, dt, kind="ExternalInput").ap()
    hT = dp("hT", [D, T], bf16)
    wqkvg = dp("wqkvg", [D, 4 * NH * DK], bf16)
    wo = dp("wo", [NH * DV, D], bf16)
    wf1 = dp("wf1", [D, DV], bf16)
    wf2 = dp("wf2", [DV, NH * NG], bf16)
    wb = dp("wb", [D, NH], bf16)
    cw = dp("cw", [NH * DK, 12], fp32)
    nega32 = dp("nega32", [32, 1], fp32)
    dtb32 = dp("dtb32", [32, 1], fp32)
    bgc = dp("bgc", [DV, NH], fp32)
    normw = dp("normw", [DV, 1], fp32)
    repl64 = dp("repl64", [64, 4 * DK], bf16)
    s64f = dp("s64f", [64, 4 * NG * C], bf16)
    oh4 = dp("oh4", [DK, 16], bf16)
    oh4b = dp("oh4b", [4, 4 * DK], bf16)
    evodb = dp("evodb", [DK, 4 * C], bf16)
    oh8 = dp("oh8", [DK, 64], bf16)
    sel8b = dp("sel8b", [8, 8 * 128], bf16)
    sc8 = dp("sc8", [8, 1], fp32)
    eps8 = dp("eps8", [8, 1], fp32)
    nmaskM = dp("nmaskM", [C, C], bf16)   # -1 strictly upper (s<t)
    maskG = dp("maskG", [C, C], bf16)     # +1 upper incl diag (s<=t)
    idbf = dp("idbf", [128, 128], bf16)
    outT = nc.dram_tensor("outT", [D, T], bf16, kind="ExternalOutput").ap()
    DBG = bool(__import__('os').environ.get('K3_DEBUG'))
    dbg = {}
    if DBG:
        for nm in ['d_qb', 'd_kb', 'd_vb', 'd_gateb', 'd_yf']:
            for m in range(4):
                dbg[f'{nm}{m}'] = nc.dram_tensor(
                    f'{nm}{m}', [128, T], bf16, kind="ExternalOutput").ap()
        dbg['d_gna'] = nc.dram_tensor('d_gna', [32, T], fp32,
                                      kind="ExternalOutput").ap()
        dbg['d_bsg'] = nc.dram_tensor('d_bsg', [4, T], bf16,
                                      kind="ExternalOutput").ap()
        dbg['d_sqs'] = nc.dram_tensor('d_sqs', [4, T], fp32,
                                      kind="ExternalOutput").ap()

    with tile.TileContext(nc) as tc, ExitStack() as ctx:
        pool = lambda name, bufs, space="SBUF": ctx.enter_context(
            tc.tile_pool(name=name, bufs=bufs, space=space))

        cons = pool("cons", 1)
        pers = pool("pers", 1)
        st = pool("st", 1)
        wk = pool("wk", 2)
        pr = pool("pr", 1, "PSUM")

        dma = nc.sync.dma_start

        adma = nc.scalar.dma_start   # second DMA queue (Act engine)

        # ---- conv weights (Act queue; needed ~25us in) ----
        cwt = []
        for m in range(4):
            t = cons.tile([128, 12], fp32, tag=f"cw{m}", name=f"cw{m}")
            adma(t[:], cw[m * 128:(m + 1) * 128, :])
            cwt.append(t)

        # single strided DMAs: (k p) x -> p k x gathers of the row-tiles
        wtiles = {}

        def load_wset(proj):
            ws = wk.tile([128, 16 * 512], bf16, tag="wset", name=f"ws{proj}",
                         bufs=2)
            for qt in range(4):
                nc.sync.dma_start(
                    ws[:, qt * 4 * 512:(qt + 1) * 4 * 512].rearrange(
                        "p (k c) -> p k c", k=4),
                    wqkvg[qt * 4 * 128:(qt + 1) * 4 * 128,
                          proj * 512:(proj + 1) * 512].rearrange(
                        "(k p) c -> p k c", k=4, p=128))
            wtiles[proj] = ws

        htab = [pers.tile([128, 16 * 512], bf16, tag=f"hth{hf}",
                          name=f"hth{hf}") for hf in range(2)]

        def load_ht(hf):
            for qt in range(4):
                nc.sync.dma_start(
                    htab[hf][:, qt * 4 * 512:(qt + 1) * 4 * 512].rearrange(
                        "p (k c) -> p k c", k=4),
                    hT[qt * 4 * 128:(qt + 1) * 4 * 128,
                       hf * 512:(hf + 1) * 512].rearrange(
                        "(k p) c -> p k c", k=4, p=128))

        def hts(k, hf):
            return htab[hf][:, k * 512:(k + 1) * 512]

        # critical stream on SP, interleaved by quarter for fast start
        for qt in range(4):
            nc.sync.dma_start(
                htab[0][:, qt * 2048:(qt + 1) * 2048].rearrange(
                    "p (k c) -> p k c", k=4),
                hT[qt * 512:(qt + 1) * 512, 0:512].rearrange(
                    "(k p) c -> p k c", k=4, p=128))
            ws0 = wtiles.setdefault(0, wk.tile([128, 16 * 512], bf16,
                                               tag="wset", name="ws0",
                                               bufs=2))
            nc.sync.dma_start(
                ws0[:, qt * 2048:(qt + 1) * 2048].rearrange(
                    "p (k c) -> p k c", k=4),
                wqkvg[qt * 512:(qt + 1) * 512, 0:512].rearrange(
                    "(k p) c -> p k c", k=4, p=128))
        load_wset(1)
        load_ht(1)

        # ---- remaining weights + consts on the Act queue ----
        wf1a = cons.tile([128, 16 * 128], bf16, tag="wf1a", name="wf1a")
        adma(wf1a[:].rearrange("p (k c) -> p k c", k=16),
             wf1[:].rearrange("(k p) c -> p k c", k=16, p=128))
        wf1t = [wf1a[:, k * 128:(k + 1) * 128] for k in range(16)]
        wba = cons.tile([128, 64], bf16, tag="wba", name="wba")
        adma(wba[:].rearrange("p (k c) -> p k c", k=16),
             wb[:].rearrange("(k p) c -> p k c", k=16, p=128))
        wbt = [wba[:, k * 4:(k + 1) * 4] for k in range(16)]

        def ctile(shape, dt, src, nm):
            t = cons.tile(shape, dt, tag=nm, name=nm)
            adma(t[:], src[:])
            return t
        wf2t = ctile([128, 32], bf16, wf2, "wf2t")
        negat = ctile([32, 1], fp32, nega32, "negat")
        dtbt = ctile([32, 1], fp32, dtb32, "dtbt")
        bgt = ctile([128, 4], fp32, bgc, "bgt")
        bgh = cons.tile([128, 4], fp32, tag="bgh", name="bgh")
        nc.vector.tensor_scalar(bgh[:], bgt[:], 0.5, None, op0=Alu.mult)
        nwt = ctile([128, 1], fp32, normw, "nwt")
        idb = ctile([128, 128], bf16, idbf, "idb")
        r64t = ctile([64, 4 * 128], bf16, repl64, "r64t")
        s64c = ctile([64, 4 * NG * C], bf16, s64f, "s64c")
        oh4t = ctile([128, 16], bf16, oh4, "oh4t")
        oh4bt = ctile([4, 4 * 128], bf16, oh4b, "oh4bt")
        evt = ctile([128, 4 * C], bf16, evodb, "evt")
        oh8t = ctile([128, 64], bf16, oh8, "oh8t")
        s8b = ctile([8, 8 * 128], bf16, sel8b, "s8b")
        sc8t = ctile([8, 1], fp32, sc8, "sc8t")
        eps8t = ctile([8, 1], fp32, eps8, "eps8t")
        nmM = ctile([128, 128], bf16, nmaskM, "nmM")
        mGt = ctile([128, 128], bf16, maskG, "mGt")
        ones32 = cons.tile([32, C], fp32, tag="ones32", name="ones32")
        nc.vector.memset(ones32[:], 1.0)
        eps4 = cons.tile([4, 1], fp32, tag="eps4", name="eps4")
        nc.vector.memset(eps4[:], EPS)
        neg4c = cons.tile([128, 4 * C], bf16, tag="neg4c", name="neg4c")
        nc.vector.memset(neg4c[:], -1.0)

        # ---- persistent activations ----
        mk = lambda p, nm: [p.tile([128, T], bf16, tag=f"{nm}{m}",
                                   name=f"{nm}{m}") for m in range(4)]
        qb, kb, vb = mk(pers, "qb"), mk(pers, "kb"), mk(pers, "vb")
        gateb, yfall = mk(pers, "gateb"), mk(pers, "yfall")
        gna32 = cons.tile([32, T], fp32, tag="gna32", name="gna32")
        bsg = cons.tile([4, T], bf16, tag="bsg", name="bsg")
        sqs = cons.tile([4, T], fp32, tag="sqs", name="sqs")
        f1b = cons.tile([128, T], bf16, tag="f1b", name="f1b")

        # ---- PSUM rings: pp x4 (proj/bca/pall/red/ssq/out), xaq x1,
        #      q32a x2, q16b x1 ----
        def pp(nm):
            return pr.tile([128, 4 * C], fp32, tag="pp", bufs=4, name=nm)

        def q32(nm):
            return pr.tile([128, 4 * C], fp32, tag="q32a", bufs=2, name=nm)

        def q16(nm):
            return pr.tile([128, 4 * C], bf16, tag="q16b", bufs=1, name=nm)

        # ---- projection pieces ----
        bnds = {}       # (proj, m) -> 3-col conv boundary

        def proj_piece(proj, m, half):
            """16 matmuls; conv projs: PSUM->xpad copy + 4 DVE taps +
            tanh-based silu (exp table only); gate: fused tanh sigmoid."""
            wt = wtiles[proj]
            sl = slice(half * 512, (half + 1) * 512)
            ps = pp(f"prj{proj}{m}{half}")
            for k in range(16):
                nc.tensor.matmul(ps[:, 0:512],
                                 wt[:, k * 512 + m * 128:k * 512
                                    + (m + 1) * 128],
                                 hts(k, half), start=(k == 0), stop=(k == 15))
            if proj == 3:
                # silu(x+bg) = (x+bg)*(0.5 + 0.5*tanh((x+bg)/2))
                xg = wk.tile([128, 512], bf16, tag="xg", name=f"xg{m}{half}",
                             bufs=2)
                nc.scalar.activation(xg[:], ps[:, 0:512], Act.Identity,
                                     bias=bgt[:, m:m + 1])
                th = wk.tile([128, 512], bf16, tag="th", name=f"th{m}{half}",
                             bufs=2)
                nc.scalar.activation(th[:], ps[:, 0:512], Act.Tanh,
                                     scale=0.5, bias=bgh[:, m:m + 1])
                sg = wk.tile([128, 512], bf16, tag="sg", name=f"sg{m}{half}",
                             bufs=2)
                nc.vector.tensor_scalar(sg[:], th[:], 0.5, 0.5, op0=Alu.mult,
                                        op1=Alu.add)
                nc.gpsimd.tensor_tensor(gateb[m][:, sl], xg[:], sg[:],
                                        op=Alu.mult)
                return
            xpad = wk.tile([128, 515], bf16, tag="xpad", name="xpad", bufs=2)
            if half == 0:
                nc.vector.memset(xpad[:, 0:3], 0.0)
            else:
                nc.vector.tensor_copy(xpad[:, 0:3], bnds[(proj, m)][:])
            nc.scalar.copy(xpad[:, 3:515], ps[:, 0:512])
            if half == 0:
                bnd = wk.tile([128, 3], bf16, tag=f"bnd{proj}{m}",
                              name=f"bnd{proj}{m}", bufs=1)
                nc.vector.tensor_copy(bnd[:], xpad[:, 512:515])
                bnds[(proj, m)] = bnd
            cwm = cwt[m]
            s = proj * 4
            a = wk.tile([128, 512], bf16, tag="acc", name=f"ac{proj}{m}{half}",
                        bufs=2)
            b2 = wk.tile([128, 512], bf16, tag="acw", name="acw", bufs=2)
            nc.vector.tensor_scalar(b2[:], xpad[:, 3:515],
                                    cwm[:, s + 3:s + 4], None, op0=Alu.mult)
            cur, nxt = b2, a
            for kk in (2, 1, 0):
                nc.vector.scalar_tensor_tensor(
                    nxt[:], xpad[:, kk:kk + 512], cwm[:, s + kk:s + kk + 1],
                    cur[:], op0=Alu.mult, op1=Alu.add)
                cur, nxt = nxt, cur
            # silu(x) = x*(0.5 + 0.5*tanh(x/2)) — stays on the exp table
            th = wk.tile([128, 512], bf16, tag="th", name=f"th{proj}{m}{half}",
                         bufs=2)
            nc.scalar.activation(th[:], a[:], Act.Tanh, scale=0.5)
            sg = wk.tile([128, 512], bf16, tag="sg", name=f"sg{proj}{m}{half}",
                         bufs=2)
            nc.vector.tensor_scalar(sg[:], th[:], 0.5, 0.5, op0=Alu.mult,
                                    op1=Alu.add)
            dst = (qb[m] if proj == 0 else kb[m] if proj == 1 else vb[m])
            nc.gpsimd.tensor_tensor(dst[:, sl], a[:], sg[:], op=Alu.mult)

        def beta_piece():
            for half in range(2):
                bps = pp(f"bps{half}")
                for k in range(16):
                    nc.tensor.matmul(bps[0:4, 0:512], wbt[k],
                                     hts(k, half),
                                     start=(k == 0), stop=(k == 15))
                # sigmoid(x) = 0.5 + 0.5*tanh(x/2) (shares the silu table)
                tb = wk.tile([4, 512], bf16, tag="tb", name=f"tb{half}")
                nc.scalar.activation(tb[:], bps[0:4, 0:512], Act.Tanh,
                                     scale=0.5)
                nc.vector.tensor_scalar(bsg[:, half * 512:(half + 1) * 512],
                                        tb[:], 0.5, 0.5, op0=Alu.mult,
                                        op1=Alu.add)

        def fgate_piece():
            """f1 projection + grouped softplus gate, both halves (Exp/Ln)."""
            for half in range(2):
                sl = slice(half * 512, (half + 1) * 512)
                ps = pp(f"f1p{half}")
                for k in range(16):
                    nc.tensor.matmul(ps[:, 0:512], wf1t[k], hts(k, half),
                                     start=(k == 0), stop=(k == 15))
                nc.scalar.copy(f1b[:, sl], ps[:, 0:512])
            for half in range(2):
                sl = slice(half * 512, (half + 1) * 512)
                gp = pp(f"gp{half}")
                nc.tensor.matmul(gp[0:32, 0:512], wf2t[:, 0:32], f1b[:, sl],
                                 start=True, stop=True)
                spe = wk.tile([32, 512], fp32, tag="spe", name=f"spe{half}", bufs=1)
                nc.scalar.activation(spe[:], gp[0:32, 0:512], Act.Exp,
                                     bias=dtbt[:, 0:1])
                sp = wk.tile([32, 512], fp32, tag="spx", name=f"sp{half}", bufs=1)
                nc.scalar.activation(sp[:], spe[:], Act.Ln,
                                     bias=ones32[:, 0:1])
                nc.vector.tensor_scalar(gna32[:, sl], sp[:], negat[:, 0:1],
                                        None, op0=Alu.mult)

        def norm_piece(half, grp):
            """l2 normalizers for pairs grp*4..grp*4+3 (q: grp 0, k: grp 1)."""
            sl = slice(half * 512, (half + 1) * 512)
            rows = slice(grp * 4, grp * 4 + 4)
            ssq = pp(f"ssq{half}{grp}")
            for i in range(4):
                p = grp * 4 + i
                src = qb[p][:, sl] if p < 4 else kb[p - 4][:, sl]
                sq = wk.tile([128, 512], bf16, tag="sq", name=f"sq{p}{half}", bufs=1)
                nc.vector.tensor_tensor(sq[:], src, src, op=Alu.mult)
                nc.tensor.matmul(ssq[0:8, 0:512], oh8t[:, p * 8:p * 8 + 8],
                                 sq[:], start=(i == 0), stop=(i == 3))
            nrm = wk.tile([8, 512], fp32, tag="nrm", name=f"nrm{half}{grp}", bufs=1)
            nc.scalar.activation(nrm[:], ssq[0:8, 0:512], Act.Ln,
                                 scale=sc8t[:, 0:1], bias=eps8t[:, 0:1])
            recb = wk.tile([8, 512], bf16, tag="recb", name=f"rec{half}{grp}")
            nc.scalar.activation(recb[:], nrm[:], Act.Exp, scale=-0.5)
            for i in range(4):
                p = grp * 4 + i
                dst = qb[p][:, sl] if p < 4 else kb[p - 4][:, sl]
                nb = pp(f"nb{p}{half}")
                nc.tensor.matmul(nb[:, 0:512], s8b[:, p * 128:(p + 1) * 128],
                                 recb[:], start=True, stop=True)
                nc.vector.tensor_tensor(dst, dst, nb[:, 0:512], op=Alu.mult)

        # ---- output projection pieces ----
        wot = {}

        def wo_load(mh):
            wot.clear()
            for k in range(4):
                t = wk.tile([128, 1024], bf16, tag=f"wo{k}", name=f"wo{k}{mh}",
                            bufs=1)
                dma(t[:], wo[k * 128:(k + 1) * 128,
                             mh * 1024:(mh + 1) * 1024])
                wot[k] = t

        def onorm_piece(c0, c1):
            w = c1 - c0
            sl = slice(c0, c1)
            nrm4 = wk.tile([4, 512], fp32, tag="nrm4", name=f"nrm4{c0}", bufs=1)
            nc.scalar.activation(nrm4[:, 0:w], sqs[:, sl], Act.Ln,
                                 scale=1.0 / DV, bias=eps4[:, 0:1])
            rst4 = wk.tile([4, 512], bf16, tag="rst4", name=f"rst4{c0}", bufs=1)
            nc.scalar.activation(rst4[:, 0:w], nrm4[:, 0:w], Act.Exp,
                                 scale=-0.5)
            for h in range(4):
                rbc = pp(f"rbc{h}_{c0}")
                nc.tensor.matmul(rbc[:, 0:w], oh4bt[:, h * 128:(h + 1) * 128],
                                 rst4[:, 0:w], start=True, stop=True)
                nc.vector.scalar_tensor_tensor(yfall[h][:, sl], yfall[h][:, sl],
                                               nwt[:, 0:1], rbc[:, 0:w],
                                               op0=Alu.mult, op1=Alu.mult)

        def om_piece(m, c0, c1):
            w = c1 - c0
            sl = slice(c0, c1)
            ps = pp(f"ops{m}_{c0}")
            mm = m % 8
            for k in range(4):
                nc.tensor.matmul(ps[:, 0:w],
                                 wot[k][:, mm * 128:(mm + 1) * 128],
                                 yfall[k][:, sl], start=(k == 0), stop=(k == 3))
            osb = wk.tile([128, 512], bf16, tag="osb", name=f"osb{m}{c0}",
                          bufs=2)
            if m % 2 == 0:
                nc.vector.tensor_copy(osb[:, 0:w], ps[:, 0:w])
            else:
                nc.scalar.copy(osb[:, 0:w], ps[:, 0:w])
            dma(outT[m * 128:(m + 1) * 128, sl], osb[:, 0:w])

        # ---- recurrence (stage_a / stage_b) ----
        Sf = [st.tile([128, 128], fp32, tag=f"Sf{h}", name=f"Sf{h}")
              for h in range(4)]
        Sb = [st.tile([128, 128], bf16, tag=f"Sb{h}", name=f"Sb{h}")
              for h in range(4)]
        for h in range(4):
            nc.vector.memset(Sf[h][:], 0.0)
            nc.vector.memset(Sb[h][:], 0.0)

        ax_store = {}

        def _a_prep(ci):
            ts = slice(ci * C, (ci + 1) * C)
            prep = pr.tile([128, 4], bf16, tag="q32a", bufs=2,
                           name=f"prep{ci}")
            nc.tensor.transpose(prep[:], bsg[:, ts], idb[0:4, 0:4])
            beta2 = wk.tile([128, 4], fp32, tag="beta2", name=f"beta2_{ci}")
            nc.scalar.copy(beta2[:], prep[:])
            cN32 = wk.tile([32, C], fp32, tag="cN32", name=f"cN32_{ci}")
            nc.vector.tensor_tensor_scan(cN32[:], ones32[:], gna32[:, ts],
                                         0.0, op0=Alu.mult, op1=Alu.add)
            c64 = wk.tile([64, C], bf16, tag="c64", name=f"c64_{ci}")
            nc.scalar.copy(c64[0:32, :], cN32[:])
            nc.vector.tensor_tensor(c64[32:64, :], cN32[:], c64[0:32, :],
                                    op=Alu.subtract)
            n64 = wk.tile([64, C], bf16, tag="n64", name=f"n64_{ci}")
            nc.gpsimd.tensor_tensor(n64[:], c64[:], neg4c[0:64, 0:C],
                                    op=Alu.mult)

            # channel decay expansion, all heads in one quad
            cfq = q32(f"cfq{ci}")
            for h in range(4):
                hs_ = slice(h * C, (h + 1) * C)
                nc.tensor.matmul(cfq[:, hs_], r64t[:, h * 128:(h + 1) * 128],
                                 c64[:], start=True, stop=True)
            nclq = wk.tile([128, 4], fp32, tag="nclq", name=f"nclq{ci}")
            for h in range(4):
                nc.vector.tensor_scalar(nclq[:, h:h + 1],
                                        cfq[:, h * C + C - 1:h * C + C],
                                        -1.0, None, op0=Alu.mult)
            bfq = wk.tile([128, 4 * C], bf16, tag="bfq", name=f"bfq{ci}", bufs=1)
            nc.scalar.activation(bfq[:], cfq[:], Act.Exp, scale=-1.0)
            kfq = wk.tile([128, 4 * C], bf16, tag="kfq", name=f"kfq{ci}", bufs=1)
            for h in range(4):
                hs_ = slice(h * C, (h + 1) * C)
                nc.scalar.activation(kfq[:, hs_], cfq[:, hs_], Act.Exp,
                                     bias=nclq[:, h:h + 1])
            bCq = wk.tile([128, 4], fp32, tag="bCq", name=f"bCq{ci}")
            nc.scalar.activation(bCq[:], nclq[:], Act.Exp)
            nbfq = wk.tile([128, 4 * C], bf16, tag="nbfq", name=f"nbfq{ci}", bufs=1)
            nc.gpsimd.tensor_tensor(nbfq[:], bfq[:], neg4c[:], op=Alu.mult)

            # decayed k/q streams (Pool)
            negWt, qtT, kend = [], [], []
            for h in range(4):
                hs_ = slice(h * C, (h + 1) * C)
                nw = wk.tile([128, C], bf16, tag=f"negWt{h}",
                             name=f"negWt{h}_{ci}")
                nc.gpsimd.tensor_tensor(nw[:], kb[h][:, ts], nbfq[:, hs_],
                                        op=Alu.mult)
                qt = wk.tile([128, C], bf16, tag=f"qtT{h}", name=f"qtT{h}_{ci}")
                nc.gpsimd.tensor_tensor(qt[:], qb[h][:, ts], bfq[:, hs_],
                                        op=Alu.mult)
                ke = wk.tile([128, C], bf16, tag=f"kend{h}",
                             name=f"kend{h}_{ci}")
                nc.gpsimd.tensor_tensor(ke[:], kb[h][:, ts], kfq[:, hs_],
                                        op=Alu.mult)
                negWt.append(nw); qtT.append(qt); kend.append(ke)

            ealls = [None] * 4

            def corr(h, srcq, mask_t, scale_col, nm, dst):
                kms = []
                for j in range(4):
                    km = wk.tile([128, C], bf16, tag="km",
                                 name=f"km{j}_{h}_{nm}_{ci}", bufs=4)
                    nc.gpsimd.tensor_tensor(km[:], kb[h][:, ts],
                                            evt[:, j * C:(j + 1) * C],
                                            op=Alu.mult)
                    kms.append(km)
                prods = []
                for half in range(2):
                    pall = pp(f"pall{nm}{h}_{half}_{ci}")
                    for j in range(4):
                        n = half * 4 + j
                        kmsk = kms[n % 4]
                        blk = 64 * (n // 4)
                        nc.tensor.matmul(
                            pall[:, j * C:(j + 1) * C],
                            kmsk[blk:blk + 64, :],
                            srcq[blk:blk + 64, ts],
                            start=True, stop=True)
                    prod = wk.tile([128, 4 * C], bf16, tag="prod",
                                   name=f"prod{nm}{h}_{half}", bufs=2)
                    easl = ealls[h][:, half * 4 * C:(half + 1) * 4 * C]
                    nc.vector.scalar_tensor_tensor(prod[:], easl, 1.0,
                                                   pall[:], op0=Alu.min,
                                                   op1=Alu.mult)
                    prods.append(prod)
                red = pp(f"red{nm}{h}_{ci}")
                for n in range(NG):
                    nc.tensor.matmul(red[:, 0:C], idb[:],
                                     prods[n // 4][:, (n % 4) * C:
                                                   (n % 4 + 1) * C],
                                     start=(n == 0), stop=(n == NG - 1))
                if scale_col is not None:
                    nc.vector.scalar_tensor_tensor(dst, red[:, 0:C],
                                                   scale_col, mask_t[:],
                                                   op0=Alu.mult, op1=Alu.mult)
                else:
                    nc.vector.tensor_tensor(dst, red[:, 0:C], mask_t[:],
                                            op=Alu.mult)

            Hq0 = wk.tile([128, 4 * C], bf16, tag="Hq", name=f"Hq{ci}_0")
            Gq = wk.tile([128, 4 * C], bf16, tag="Gq", name=f"Gq{ci}")
            ax = dict(ts=ts, beta2=beta2, bCq=bCq, negWt=negWt,
                      qtT=qtT, kend=kend, Hq0=Hq0, Gq=Gq, ci=ci,
                      ealls=ealls, corr=corr, c64=c64, n64=n64)
            ax_store[ci] = ax

        def _a_eall(ci, h):
            ax = ax_store[ci]
            c64, n64 = ax['c64'], ax['n64']
            base = h * NG * C
            ea = wk.tile([128, NG * C], bf16, tag="eall",
                         name=f"eall{h}_{ci}", bufs=2)
            for half in range(2):
                bca = pp(f"bca{h}_{half}_{ci}")
                nc.tensor.matmul(bca[:],
                                 n64[:],
                                 s64c[:, base + half * 512:base + half * 512
                                      + 512],
                                 start=True, stop=False)
                for j in range(4):
                    n = half * 4 + j
                    nc.tensor.matmul(bca[:, j * C:(j + 1) * C],
                                     s64c[:, base + n * 128:base
                                          + (n + 1) * 128],
                                     c64[:], start=False, stop=(j == 3))
                nc.scalar.activation(
                    ea[:, half * 4 * C:(half + 1) * 4 * C], bca[:],
                    Act.Exp, scale=-1.0)
            ax['ealls'][h] = ea

        def _a_corr(ci, h, which):
            ax = ax_store[ci]
            if which == 'M':
                ax['corr'](h, kb[h], nmM, ax['beta2'][:, h:h + 1], "M",
                           ax['Hq0'][:, h * C:(h + 1) * C])
            else:
                ax['corr'](h, qb[h], mGt, None, "G",
                           ax['Gq'][:, h * C:(h + 1) * C])

        def sa_pieces(ci):
            ps = [lambda ci=ci: _a_prep(ci)]
            for h in range(4):
                ps.append(lambda ci=ci, h=h: _a_eall(ci, h))
                ps.append(lambda ci=ci, h=h: _a_corr(ci, h, 'M'))
                ps.append(lambda ci=ci, h=h: _a_corr(ci, h, 'G'))
            return ps

        # ---- work queue ----
        WQ = []
        wq_pos = [0]
        PUMP_CAP = [None]

        def pump(n):
            e = min(wq_pos[0] + n, len(WQ))
            if PUMP_CAP[0] is not None:
                e = min(e, PUMP_CAP[0])
            while wq_pos[0] < e:
                WQ[wq_pos[0]]()
                wq_pos[0] += 1

        def drain_until(mark):
            while wq_pos[0] < mark:
                WQ[wq_pos[0]]()
                wq_pos[0] += 1

        def stage_b(ci):
            pmp = pump
            ax = ax_store[ci]
            ts = ax['ts']
            beta2, bCq = ax['beta2'], ax['bCq']
            negWt, qtT, kend = ax['negWt'], ax['qtT'], ax['kend']
            Hq, Gq = ax['Hq0'], ax['Gq']

            xaq = pr.tile([128, 4 * C], fp32, tag="xaq", bufs=1,
                          name=f"xaq{ci}")
            xaccs = [xaq[:, h * C:(h + 1) * C] for h in range(4)]
            for h in range(4):
                nc.tensor.matmul(xaccs[h], vb[h][:, ts], idb[:],
                                 start=True, stop=False)
                nc.tensor.matmul(xaccs[h], negWt[h][:], Sb[h][:],
                                 start=False, stop=True)
            for lev in range(NLEV):
                pmp(3)
                last = (lev == NLEV - 1)
                xbq = wk.tile([128, 4 * C], bf16, tag="xbq",
                              name=f"xbq{ci}_{lev}", bufs=1)
                nc.scalar.copy(xbq[:], xaq[:])
                xaq = pr.tile([128, 4 * C], fp32, tag="xaq", bufs=1,
                              name=f"xaq{ci}_{lev}")
                xaccs = [xaq[:, h * C:(h + 1) * C] for h in range(4)]
                for h in range(4):
                    hs_ = slice(h * C, (h + 1) * C)
                    nc.tensor.matmul(xaccs[h], idb[:], xbq[:, hs_],
                                     start=True, stop=False)
                    nc.tensor.matmul(xaccs[h], Hq[:, hs_], xbq[:, hs_],
                                     start=False, stop=True)
                if not last:
                    htrq = q16(f"htr{ci}_{lev}")
                    for h in range(4):
                        nc.tensor.transpose(htrq[:, h * C:(h + 1) * C],
                                            Hq[:, h * C:(h + 1) * C],
                                            idb[:])
                    htsq = wk.tile([128, 4 * C], bf16, tag="htsq",
                                   name=f"htsq{ci}_{lev}", bufs=1)
                    nc.scalar.copy(htsq[:], htrq[:])
                    h2q = q32(f"h2q{ci}_{lev}")
                    for h in range(4):
                        hs_ = slice(h * C, (h + 1) * C)
                        nc.tensor.matmul(h2q[:, hs_], htsq[:, hs_],
                                         Hq[:, hs_], start=True, stop=True)
                    Hq = wk.tile([128, 4 * C], bf16, tag="Hq",
                                 name=f"Hq{ci}_{lev + 1}")
                    nc.scalar.copy(Hq[:], h2q[:])

            ubs = []
            for h in range(4):
                ub = wk.tile([128, C], bf16, tag=f"ub{h}", name=f"ub{h}_{ci}")
                nc.vector.tensor_scalar(ub[:], xaccs[h], beta2[:, h:h + 1],
                                        None, op0=Alu.mult)
                ubs.append(ub)
            otq = q32(f"otq{ci}")
            ktq = q16(f"ktq{ci}")
            for h in range(4):
                hs_ = slice(h * C, (h + 1) * C)
                nc.tensor.matmul(otq[:, hs_], Sb[h][:], qtT[h][:],
                                 start=True, stop=False)
                nc.tensor.matmul(otq[:, hs_], ubs[h][:], Gq[:, hs_],
                                 start=False, stop=True)
                nc.tensor.transpose(ktq[:, hs_], kend[h][:], idb[:])
            pmp(2)
            ktsq = wk.tile([128, 4 * C], bf16, tag="ktsq", name=f"ktsq{ci}", bufs=1)
            nc.scalar.copy(ktsq[:], ktq[:])
            suq = q32(f"suq{ci}")
            for h in range(4):
                hs_ = slice(h * C, (h + 1) * C)
                nc.tensor.matmul(suq[:, hs_], ktsq[:, hs_], ubs[h][:],
                                 start=True, stop=True)
                nc.vector.scalar_tensor_tensor(Sf[h][:], Sf[h][:],
                                               bCq[:, h:h + 1], suq[:, hs_],
                                               op0=Alu.mult, op1=Alu.add)
                nc.scalar.copy(Sb[h][:], Sf[h][:])
            sspq = None
            for h in range(4):
                hs_ = slice(h * C, (h + 1) * C)
                yf = yfall[h]
                nc.vector.tensor_tensor(yf[:, ts], gateb[h][:, ts],
                                        otq[:, hs_], op=Alu.mult)
                ysq = wk.tile([128, C], bf16, tag=f"ysq{h}", name=f"ysq{h}_{ci}")
                nc.gpsimd.tensor_tensor(ysq[:], yf[:, ts], yf[:, ts],
                                        op=Alu.mult)
                if h == 0:
                    sspq = pr.tile([128, 4 * C], fp32, tag="xaq", bufs=1,
                                   name=f"ssp{ci}")
                nc.tensor.matmul(sspq[0:4, 0:C], oh4t[:, 4 * h:4 * h + 4],
                                 ysq[:], start=(h == 0), stop=(h == 3))
                if h == 3:
                    nc.scalar.copy(sqs[:, ts], sspq[0:4, 0:C])
            pmp(3)

        # =================== EMISSION ===================
        # Segment A: half-0 projections + gates + norms, then chunk 0 prep.
        for m in range(4):
            proj_piece(0, m, 0)
        load_wset(1)
        for m in range(4):
            proj_piece(1, m, 0)
        load_wset(2)
        for m in range(4):
            proj_piece(2, m, 0)
        load_wset(3)
        for m in range(4):
            proj_piece(3, m, 0)
        beta_piece()
        fgate_piece()
        norm_piece(0, 0)
        norm_piece(0, 1)
        for p in sa_pieces(0):
            p()

        # Work queue: half-1 projections + chunk preps + output pieces.
        marks = {}
        for m in range(4):
            WQ.append(lambda m=m: (load_wset(0) if m == 0 else None,
                                   proj_piece(0, m, 1)))
        for p in sa_pieces(1):
            WQ.append(p)
        marks[1] = len(WQ)
        WQ.append(lambda: load_wset(1))
        for m in range(4):
            WQ.append(lambda m=m: proj_piece(1, m, 1))
        for p in sa_pieces(2):
            WQ.append(p)
        marks[2] = len(WQ)
        WQ.append(lambda: load_wset(2))
        for m in range(4):
            WQ.append(lambda m=m: proj_piece(2, m, 1))
        for p in sa_pieces(3):
            WQ.append(p)
        marks[3] = len(WQ)
        WQ.append(lambda: load_wset(3))
        for m in range(4):
            WQ.append(lambda m=m: proj_piece(3, m, 1))
        WQ.append(lambda: norm_piece(1, 0))
        WQ.append(lambda: norm_piece(1, 1))
        for p in sa_pieces(4):
            WQ.append(p)
        marks[4] = len(WQ)
        WQ.append(lambda: wo_load(0))
        WQ.append(lambda: onorm_piece(0, 512))
        for m in range(8):
            WQ.append(lambda m=m: om_piece(m, 0, 512))
        for p in sa_pieces(5):
            WQ.append(p)
        marks[5] = len(WQ)
        WQ.append(lambda: wo_load(1))
        for m in range(8, 16):
            WQ.append(lambda m=m: om_piece(m, 0, 512))
        for p in sa_pieces(6):
            WQ.append(p)
        marks[6] = len(WQ)
        for p in sa_pieces(7):
            WQ.append(p)
        marks[7] = len(WQ)
        WQ.append(lambda: wo_load(0))
        WQ.append(lambda: onorm_piece(512, 896))
        for m in range(8):
            WQ.append(lambda m=m: om_piece(m, 512, 896))
        WQ.append(lambda: wo_load(1))
        for m in range(8, 16):
            WQ.append(lambda m=m: om_piece(m, 512, 896))
        PUMP_CAP[0] = marks['tail'] = len(WQ)
        WQ.append(lambda: wo_load(0))
        WQ.append(lambda: onorm_piece(896, 1024))
        for m in range(8):
            WQ.append(lambda m=m: om_piece(m, 896, 1024))
        WQ.append(lambda: wo_load(1))
        for m in range(8, 16):
            WQ.append(lambda m=m: om_piece(m, 896, 1024))
        marks[8] = len(WQ)

        for ci in range(NCH):
            if ci > 0:
                drain_until(marks[ci])
            stage_b(ci)
        PUMP_CAP[0] = None
        drain_until(marks[8])
        if DBG:
            for m in range(4):
                dma(dbg[f'd_qb{m}'], qb[m][:])
                dma(dbg[f'd_kb{m}'], kb[m][:])
                dma(dbg[f'd_vb{m}'], vb[m][:])
                dma(dbg[f'd_gateb{m}'], gateb[m][:])
                dma(dbg[f'd_yf{m}'], yfall[m][:])
            dma(dbg['d_gna'], gna32[:])
            dma(dbg['d_bsg'], bsg[:])
            dma(dbg['d_sqs'], sqs[:])

    nc.compile()
    return nc


def _prep_inputs(inputs):
    f32 = np.float32
    hs = np.asarray(inputs['hidden_states'], f32)
    maps = []
    tri = np.tril(np.ones((C, C), f32))
    nmaskM = (-(1.0 - tri)).astype(BF)                      # -1 strictly upper
    maskG = (1.0 - tri + np.eye(C, dtype=f32)).astype(BF)   # +1 upper incl diag
    repl = np.zeros((NG, DK), f32)
    for n in range(NG):
        repl[n, n * GG:(n + 1) * GG] = 1.0
    sel8 = np.zeros((NG, NG * 128), f32)
    for n in range(NG):
        sel8[n, n * 128:(n + 1) * 128] = 1.0
    # per-head selectors on a 64-partition layout: rows 8h+g (hi) and
    # 32+8h+g (lo)
    s64 = np.zeros((64, 4 * NG * 128), f32)
    repl64 = np.zeros((64, 4 * DK), f32)
    for h in range(NH):
        s64[8 * h:8 * h + 8, h * 1024:(h + 1) * 1024] = sel8
        s64[32 + 8 * h:32 + 8 * h + 8, h * 1024:(h + 1) * 1024] = sel8
        repl64[8 * h:8 * h + 8, h * DK:(h + 1) * DK] = repl
        repl64[32 + 8 * h:32 + 8 * h + 8, h * DK:(h + 1) * DK] = repl
    oh8 = np.zeros((DK, 64), f32)
    for i in range(8):
        oh8[:, i * 8 + i] = 1.0
    oh4 = np.zeros((DK, 16), f32)
    for i in range(4):
        oh4[:, i * 4 + i] = 1.0
    oh4b = np.zeros((4, 4 * DK), f32)
    for i in range(4):
        oh4b[i, i * 128:(i + 1) * 128] = 1.0
    evodb = np.zeros((DK, 4 * C), f32)
    for cc in range(DK):
        evodb[cc, ((cc // GG) % 4) * C:(((cc // GG) % 4) + 1) * C] = 1.0
    ident = np.eye(128, dtype=f32)
    hTs = [np.ascontiguousarray(hs[b].T).astype(BF) for b in range(B)]
    for c in range(8):
        b, hg = c // 4, c % 4
        cols = slice(hg * NH * DK, (hg + 1) * NH * DK)
        gcols = slice(hg * NH * NG, (hg + 1) * NH * NG)
        hcols = slice(hg * NH, (hg + 1) * NH)
        Alog = np.asarray(inputs['A_log'], f32)[hcols]
        nega32 = np.repeat(np.exp(Alog), NG)[:, None]       # rows 8h+g
        dtb32 = np.asarray(inputs['dt_bias'], f32)[gcols].reshape(
            NH, NG).reshape(32)[:, None]
        m = {
            'hT': hTs[b],
            'wqkvg': np.ascontiguousarray(np.concatenate(
                [np.asarray(inputs['Wq'], f32)[:, cols],
                 np.asarray(inputs['Wk'], f32)[:, cols],
                 np.asarray(inputs['Wv'], f32)[:, cols],
                 np.asarray(inputs['Wg'], f32)[:, cols]], 1)).astype(BF),
            'wo': np.asarray(inputs['Wo'], f32)[cols, :].astype(BF),
            'wf1': np.asarray(inputs['Wf1'], f32).astype(BF),
            'wf2': np.asarray(inputs['Wf2'], f32)[:, gcols].astype(BF),
            'wb': np.asarray(inputs['Wb'], f32)[:, hcols].astype(BF),
            'cw': np.ascontiguousarray(np.concatenate(
                [np.asarray(inputs['conv_q'], f32)[cols],
                 np.asarray(inputs['conv_k'], f32)[cols],
                 np.asarray(inputs['conv_v'], f32)[cols]], 1)),
            'nega32': np.ascontiguousarray(nega32, f32),
            'dtb32': np.ascontiguousarray(dtb32, f32),
            'bgc': np.ascontiguousarray(
                np.asarray(inputs['bg'], f32)[cols].reshape(NH, DV).T),
            'normw': np.ascontiguousarray(
                np.asarray(inputs['norm_w'], f32)[:, None]),
            'repl64': repl64.astype(BF),
            's64f': s64.astype(BF),
            'sel8b': sel8.astype(BF),
            'oh4': oh4.astype(BF),
            'oh4b': oh4b.astype(BF),
            'evodb': evodb.astype(BF),
            'oh8': oh8.astype(BF),
            'sc8': np.array([[1.0 / SCALE ** 2]] * 4 + [[1.0]] * 4, f32),
            'eps8': np.array([[1e-6 / SCALE ** 2]] * 4 + [[1e-6]] * 4, f32),
            'nmaskM': nmaskM,
            'maskG': maskG,
            'idbf': ident.astype(BF),
        }
        maps.append(m)
    return maps


def kernel(**inputs):
    from concourse.bass_utils import run_bass_kernel_spmd
    if 'nc' not in _CACHE:
        _CACHE['nc'] = _build()
    nc = _CACHE['nc']
    maps = _prep_inputs(inputs)
    res = run_bass_kernel_spmd(nc, maps, list(range(8))).results
    out = np.zeros((B, T, D), np.float32)
    for c in range(8):
        out[c // 4] += np.asarray(res[c]['outT'], np.float32).T
    return out
